# revision 1
# baseline (speedup 1.0000x reference)
"""NuFFT forward (KbNufft-style) Trainium2 Bass kernel.

Strategy:
  - Visibilities only touch |k| <= ~400 of the 2048-point oversampled grid
    (UMAX bound), so each of the 8 cores computes a 105-row x 804-col slab of
    the spectrum via DFT matmuls (apodization folded into the DFT constants):
        slab = Fv_rows . (cube/apod) . Fu_cols^T
  - Visibilities are sharded across cores by their v-row bin, so every
    core's slab fully covers its own visibilities' 6x6 KB footprints.
  - The slab is stored channel-interleaved in DRAM (row = [col][chan][re/im],
    padded to a 256B-multiple row stride); the 6x6 interpolation becomes bulk
    gpsimd.dma_gather calls (256B descriptors; visibilities binned by
    col-offset residue j0%8 so int16 indices address 64-f32-aligned starts
    from an 8*r f32 base offset), then a DVE multiply-reduce against
    host-precomputed 48-tap weight products (6 rows x 8 cols, last 2 zero).
"""
import os
import sys

for _p in ("/opt/trn_rl_repo",):
    if _p not in sys.path and os.path.isdir(_p):
        sys.path.insert(0, _p)

import numpy as np

# ---- problem constants (must match reference.py) ----
NCH = 4
NPIX = 1024
NVIS = 200_000
G = 2048
J = 6
OSF = 2
CELL_ARCSEC = 0.005
DL = CELL_ARCSEC * np.pi / (180.0 * 3600.0)
BETA = float(np.pi * np.sqrt((J / OSF) ** 2 * (OSF - 0.5) ** 2 - 0.8))

# ---- sharding geometry ----
N_CORES = 8
P = 128                      # SBUF partitions
ROW_LO_ALL = -398            # min possible m0 (floor of t), |t| < 397.2
ROWS_PER_CORE = 100
R_ROWS = ROWS_PER_CORE + 5   # 105 slab rows per core (footprint halo)
KU = 804                     # slab cols, c'_u in [-401, 403)
COL_BASE = -401
ROW_F32 = 6464               # padded slab row: 808 cols * 8 = 101*256B stride
STRIP = 408                  # stage-2 ku strip width (2 overlapping strips)
SOFF = (0, 396)              # strip col offsets; windows never straddle
N1 = 3 * R_ROWS + 1          # stage-1 rhs width (f32r needs even N)

NRES = 8                     # col-residue streams per strip
R_SLOTS = 14                 # vis slots per partition per (strip, residue)
N_STREAMS = 2 * NRES         # 16 gather streams
V_SLOTS = N_STREAMS * R_SLOTS        # 224 output rows per partition
GSTRIDE = 3328               # f32 per strip-grid row (52*256B stride)
GBLK = GSTRIDE // 64         # 52 64-f32 blocks per row
DESC_PER_S = P * R_SLOTS * J         # 10752 descriptors per stream
CALL_IDX = 1024                      # dma_gather ring capacity per call
IDXCOLS_S = DESC_PER_S // 16          # 672 int16 cols per stream
GROWS2 = (R_ROWS * GSTRIDE - 56) // 64  # 5459 64-f32 rows addressable

C1 = np.float32(1000.0 * 2.0 * np.pi * DL)   # klambda -> rad/pixel
C2 = np.float32(G / (2.0 * np.pi))           # rad/pixel -> grid coord

_NC_CACHE = {}


def _matmul_dtype():
    return os.environ.get("NUFFT_MM_DTYPE", "float32r")


def build_nc():
    """Build the SPMD Bass program (same program for all 8 cores)."""
    key = _matmul_dtype()
    if key in _NC_CACHE:
        return _NC_CACHE[key]

    import concourse.bacc as bacc
    import concourse.mybir as mybir
    import concourse.tile as tile
    from concourse import library_config
    from contextlib import ExitStack

    f32 = mybir.dt.float32
    i16 = mybir.dt.int16
    mm_dt = getattr(mybir.dt, key)

    nc = bacc.Bacc("TRN2", target_bir_lowering=False, debug=False)

    cube_d = nc.dram_tensor("cube", (NCH, NPIX, NPIX), mm_dt, kind="ExternalInput")
    cvt_d = nc.dram_tensor("cvt", (P, 8, N1), mm_dt, kind="ExternalInput")
    cut_d = nc.dram_tensor("cut", (P, 8, KU), mm_dt, kind="ExternalInput")
    sut_d = nc.dram_tensor("sut", (P, 8, KU), mm_dt, kind="ExternalInput")
    gidx_d = nc.dram_tensor("gidx", (P, N_STREAMS * IDXCOLS_S), i16,
                            kind="ExternalInput")
    w48_d = nc.dram_tensor("w48", (P, V_SLOTS, 48), f32, kind="ExternalInput")
    out_d = nc.dram_tensor("vis_out", (P, V_SLOTS, 8), f32,
                           kind="ExternalOutput")
    grid_d = [nc.dram_tensor(f"gridscratch{i}", (R_ROWS, GSTRIDE), f32)
              for i in range(2)]


    with tile.TileContext(nc) as tc:
        with ExitStack() as s12:
            # one lifetime for all pools: stage-3 tiles must NOT reuse
            # stage-1/2 SBUF zones, else their allocations pick up deps on
            # the tall/grid release (forcing gathers to wait for strip 1)
            const_pool = s12.enter_context(tc.tile_pool(name="const", bufs=1))
            cube_pool = s12.enter_context(tc.tile_pool(name="cube", bufs=3))
            tpool = s12.enter_context(tc.tile_pool(name="tmats", bufs=1))
            cpool = s12.enter_context(tc.tile_pool(name="cstream", bufs=4))
            psum_pool = s12.enter_context(
                tc.tile_pool(name="ps", bufs=8, space="PSUM"))

            cvt_sb = const_pool.tile([P, 8, N1], mm_dt)
            nc.sync.dma_start(cvt_sb[:], cvt_d[:])

            # T storage: (p, chan, term[T1,T2,negT1], xc, r)
            tall = tpool.tile([P, NCH, 3, 8, R_ROWS], mm_dt)

            # ---- stage 1: T^T = cube^T . cvt (accumulate over y chunks) ----
            for c in range(NCH):
                ps = [psum_pool.tile([P, N1], f32, tag="ps",
                                     name=f"ps1_{c}_{i}") for i in range(8)]
                for yc in range(8):
                    cb = cube_pool.tile([P, NPIX], mm_dt, tag="cube")
                    nc.sync.dma_start(cb[:], cube_d[c, yc * P:(yc + 1) * P, :])
                    for xt in range(8):
                        nc.tensor.matmul(
                            ps[xt][:],
                            lhsT=cb[:, xt * P:(xt + 1) * P],
                            rhs=cvt_sb[:, yc, :],
                            start=(yc == 0),
                            stop=(yc == 7),
                        )
                for xt in range(8):
                    for term in range(3):
                        nc.vector.tensor_copy(
                            tall[:, c, term, xt, :],
                            ps[xt][:, term * R_ROWS:(term + 1) * R_ROWS],
                        )

            # ---- stage 2: slab = T . [cut|sut], interleave, DMA to DRAM ----
            grid_sb = tpool.tile([P, KU * 8], f32)
            gv = grid_sb[:].rearrange("p (col e) -> p col e", e=8)
            zpad = cpool.tile([P, GSTRIDE - STRIP * 8], f32, tag="zpad")
            nc.gpsimd.memset(zpad[:], 0.0)
            for strip in range(2):
                off = SOFF[strip]
                ps2 = [psum_pool.tile([P, STRIP], f32, tag="ps",
                                      name=f"ps2_{strip}_{i}")
                       for i in range(8)]  # (c, re/im) -> ps2[c*2+e]
                for xc in range(8):
                    cu = cpool.tile([P, STRIP], mm_dt, tag="cu")
                    nc.sync.dma_start(
                        cu[:], cut_d[:, xc, off:off + STRIP])
                    su = cpool.tile([P, STRIP], mm_dt, tag="su")
                    nc.sync.dma_start(
                        su[:], sut_d[:, xc, off:off + STRIP])
                    for c in range(NCH):
                        t1 = tall[:, c, 0, xc, :]
                        t2 = tall[:, c, 1, xc, :]
                        nt1 = tall[:, c, 2, xc, :]
                        cuv = cu[:]
                        suv = su[:]
                        # re = T1.cu + T2.su ; im = T2.cu + (-T1).su
                        nc.tensor.matmul(ps2[c * 2][:R_ROWS, :], lhsT=t1,
                                         rhs=cuv, start=(xc == 0), stop=False)
                        nc.tensor.matmul(ps2[c * 2][:R_ROWS, :], lhsT=t2,
                                         rhs=suv, start=False, stop=(xc == 7))
                        nc.tensor.matmul(ps2[c * 2 + 1][:R_ROWS, :], lhsT=t2,
                                         rhs=cuv, start=(xc == 0), stop=False)
                        nc.tensor.matmul(ps2[c * 2 + 1][:R_ROWS, :], lhsT=nt1,
                                         rhs=suv, start=False, stop=(xc == 7))
                skip = 0 if strip == 0 else (SOFF[0] + STRIP) - SOFF[1]
                for c in range(NCH):
                    for e in range(2):
                        nc.vector.tensor_copy(
                            gv[:R_ROWS, off + skip:off + STRIP, c * 2 + e],
                            ps2[c * 2 + e][:R_ROWS, skip:],
                        )
                # ship this strip's slab so its gathers can start early
                nc.sync.dma_start(
                    grid_d[strip][:, :STRIP * 8],
                    grid_sb[:R_ROWS, off * 8:(off + STRIP) * 8])
                nc.sync.dma_start(grid_d[strip][:, STRIP * 8:],
                                  zpad[:R_ROWS, :])

            # ---- stage 3: residue-binned dma_gather + weighted reduce ----
            ipool = s12.enter_context(tc.tile_pool(name="interp", bufs=3))
            opool = s12.enter_context(tc.tile_pool(name="outp", bufs=1))

            nc.gpsimd.load_library(library_config.mlp)
            ov = opool.tile([P, V_SLOTS, 8], f32)
            flats = [grid_d[i][:, :].flatten() for i in range(2)]
            for st in range(N_STREAMS):
                sgrid, r = st // NRES, st % NRES
                view_r = flats[sgrid][8 * r: 8 * r + GROWS2 * 64].rearrange(
                    "(n e) -> n e", e=64)
                idxr = ipool.tile([P, IDXCOLS_S], i16, tag="idx",
                                  name=f"idx_{st}")
                nc.sync.dma_start(
                    idxr[:], gidx_d[:, st * IDXCOLS_S:(st + 1) * IDXCOLS_S])
                w = ipool.tile([P, R_SLOTS * 48], f32, tag="w", name=f"w_{st}")
                nc.sync.dma_start(
                    w[:],
                    w48_d[:, st * R_SLOTS:(st + 1) * R_SLOTS, :].rearrange(
                        "p v t -> p (v t)"))
                g = ipool.tile([P, R_SLOTS * J, 64], f32, tag="g",
                               name=f"g_{st}")
                done = 0
                k = 0
                while done < DESC_PER_S:
                    n_idx = min(CALL_IDX, DESC_PER_S - done)
                    nc.gpsimd.dma_gather(
                        out_ap=g[:, done // P:(done + n_idx) // P, :],
                        in_ap=view_r,
                        idxs_ap=idxr[:, done // 16:(done + n_idx) // 16],
                        num_idxs=n_idx,
                        num_idxs_reg=n_idx,
                        elem_size=64,
                        elem_step=64,
                    )
                    done += n_idx
                    k += 1
                # multiply by weights (broadcast over chan/reim)
                gw = g[:].rearrange("p t (col e) -> p (t col) e", e=8)
                wb = w[:].unsqueeze(2).to_broadcast([P, R_SLOTS * 48, 8])
                nc.vector.tensor_tensor(
                    out=gw, in0=gw, in1=wb, op=mybir.AluOpType.mult)
                # reduce over the 48 (6 rows x 8 cols, 2 zero) taps
                rv = g[:].rearrange(
                    "p (v i) (col e) -> p v e (i col)", v=R_SLOTS, i=J, e=8)
                nc.vector.tensor_reduce(
                    out=ov[:, st * R_SLOTS:(st + 1) * R_SLOTS, :],
                    in_=rv,
                    axis=mybir.AxisListType.X,
                    op=mybir.AluOpType.add,
                )
            nc.sync.dma_start(out_d[:], ov[:])

    nc.compile()
    _NC_CACHE[key] = nc
    return nc


def _apod1d():
    f = np.arange(NPIX, dtype=np.float64) / G
    z = np.pi * J * f
    s = np.sqrt(BETA * BETA - z * z)
    return J * np.sinh(s) / s  # [NPIX] float64


def _interp_host(k):
    """Match reference _interp_coords index/weight math in f32."""
    t = (k.astype(np.float32) * C1) * C2
    m0 = np.floor(t).astype(np.int32)
    offs = np.arange(J, dtype=np.int32) - (J // 2 - 1)
    d = t[:, None] - (m0[:, None] + offs).astype(np.float32)
    w = np.i0(BETA * np.sqrt(np.maximum(0.0, 1.0 - (2.0 * d / J) ** 2)))
    return t, m0, w.astype(np.float32)


def host_prep(cube, uu, vv):
    """Returns (in_maps, meta, phase) for the 8 cores."""
    mmkey = _matmul_dtype()
    if mmkey == "bfloat16":
        import ml_dtypes
        mmnp = ml_dtypes.bfloat16
    else:
        mmnp = np.float32
    cube = np.ascontiguousarray(np.asarray(cube, dtype=np.float32)).astype(mmnp)
    uu = np.asarray(uu, dtype=np.float32)
    vv = np.asarray(vv, dtype=np.float32)

    s1 = _apod1d()
    y = np.arange(NPIX, dtype=np.float64)

    # u-direction DFT constants (same for all cores)
    kj = np.arange(KU, dtype=np.float64) + COL_BASE
    ang_u = 2.0 * np.pi * np.outer(y, kj) / G
    cut = (np.cos(ang_u) / s1[:, None]).astype(np.float32)
    sut = (np.sin(ang_u) / s1[:, None]).astype(np.float32)
    cut = np.ascontiguousarray(cut.reshape(8, P, KU).transpose(1, 0, 2)).astype(mmnp)
    sut = np.ascontiguousarray(sut.reshape(8, P, KU).transpose(1, 0, 2)).astype(mmnp)

    tu, m0u, wu = _interp_host(uu)
    tv, m0v, wv = _interp_host(vv)
    assert m0u.min() >= ROW_LO_ALL and m0u.max() < ROW_LO_ALL + 8 * ROWS_PER_CORE
    assert m0v.min() >= ROW_LO_ALL and m0v.max() < ROW_LO_ALL + 8 * ROWS_PER_CORE

    core_of = (m0v - ROW_LO_ALL) // ROWS_PER_CORE
    j0 = m0u - 2 - COL_BASE        # window start col within slab, [1, 796]
    sgrid = (j0 > 400).astype(np.int64)
    colp = j0 - 396 * sgrid        # col within strip grid, [1,400] or [5,407]
    res = colp % NRES
    q = colp // NRES               # 64-f32 block within strip row, [0, 50]
    w48 = np.zeros((len(uu), J, 8), dtype=np.float32)
    w48[:, :, :J] = wv[:, :, None] * wu[:, None, :]

    in_maps = []
    meta = []
    for k in range(N_CORES):
        row_lo = ROW_LO_ALL + ROWS_PER_CORE * k
        gidx = np.zeros((P, N_STREAMS * IDXCOLS_S), dtype=np.int16)
        w48k = np.zeros((P, V_SLOTS, 48), dtype=np.float32)
        meta_k = []
        for st in range(N_STREAMS):
            sg, r = st // NRES, st % NRES
            order = np.where((core_of == k) & (sgrid == sg) & (res == r))[0]
            n = len(order)
            assert n <= P * R_SLOTS, f"core {k} stream {st} overflow: {n}"
            sl = np.arange(n)
            pp = sl % P
            vs = sl // P
            lrow = (m0v[order] - row_lo).astype(np.int64)   # [0, 100)
            vals = (lrow[:, None] + np.arange(J)[None, :]) * GBLK \
                + q[order, None].astype(np.int64)           # [n, J] <= 5458
            # descriptor t = (v*6+i)*128 + p ; idx A[t%16, t//16]
            t = (vs[:, None] * J + np.arange(J)[None, :]) * P + pp[:, None]
            block = np.zeros((16, IDXCOLS_S), dtype=np.int16)
            block[(t % 16).ravel(), (t // 16).ravel()] = vals.astype(
                np.int16).ravel()
            gidx[:, st * IDXCOLS_S:(st + 1) * IDXCOLS_S] = np.tile(block,
                                                                   (8, 1))
            w48k[pp, st * R_SLOTS + vs, :] = w48[order].reshape(n, 48)
            meta_k.append((order, pp, st * R_SLOTS + vs))
        # v-direction DFT constants for this core's rows
        kr = np.arange(R_ROWS, dtype=np.float64) + (row_lo - 2)
        ang_v = 2.0 * np.pi * np.outer(y, kr) / G
        blk = np.zeros((NPIX, 3 * R_ROWS + 1), dtype=np.float32)
        cosb = np.cos(ang_v) / s1[:, None]
        sinb = np.sin(ang_v) / s1[:, None]
        blk[:, 0 * R_ROWS:1 * R_ROWS] = cosb
        blk[:, 1 * R_ROWS:2 * R_ROWS] = -sinb
        blk[:, 2 * R_ROWS:3 * R_ROWS] = -cosb
        cvt = np.ascontiguousarray(
            blk.reshape(8, P, 3 * R_ROWS + 1).transpose(1, 0, 2)).astype(mmnp)

        in_maps.append({
            "cube": cube,
            "cvt": cvt,
            "cut": cut,
            "sut": sut,
            "gidx": gidx,
            "w48": w48k,
        })
        meta.append(meta_k)

    kv = vv * C1
    ku_ = uu * C1
    phase = np.exp(1j * (kv + ku_) * np.float32(NPIX / 2.0)).astype(np.complex64)
    return in_maps, meta, phase


def assemble(results, meta, phase):
    out = np.zeros((NCH, NVIS), dtype=np.complex64)
    for k in range(N_CORES):
        arr = results[k]["vis_out"].reshape(P, V_SLOTS, NCH, 2)
        for order, pp, rows in meta[k]:
            vals = arr[pp, rows]  # [n, NCH, 2]
            out[:, order] = (vals[..., 0] + 1j * vals[..., 1]).T
    return out * phase[None, :]


def kernel(cube, uu, vv):
    from concourse.bass_utils import run_bass_kernel_spmd

    nc = build_nc()
    in_maps, meta, phase = host_prep(cube, uu, vv)
    br = run_bass_kernel_spmd(
        nc, in_maps, list(range(N_CORES)),
        trace=bool(int(os.environ.get("NUFFT_TRACE", "0"))),
    )
    if br.exec_time_ns is not None:
        print(f"HW exec time: {br.exec_time_ns} ns")
    kernel.last_result = br
    return assemble(br.results, meta, phase)



# revision 6
# speedup vs baseline: 1.4127x; 1.4127x over previous
"""NuFFT forward (KbNufft-style) Trainium2 Bass kernel, v2.

Strategy (per core; vis sharded by v-row bin across 8 cores):
  - Stages 1-2 compute a 105-row x 804-col slab of the oversampled spectrum
    via partial-DFT matmuls (f32r, apodization folded into the constants):
        T = cube^T . [cos | -sin]   (per channel, 256-wide rhs for f32r rate)
        slab_re/im = T . [cut|sut]  (two 408-col strips)
  - The slab (scaled by 2^23) is stored fp16 and expanded into a 6-row
    replicated layout B[r][c] = rows r..r+5 of col c (48B cells) using 5
    shift-matmuls + Act/DVE interleave copies, then DMA'd to DRAM.
  - Each visibility's whole 6x6 KB window then becomes ONE 768B gather
    descriptor (8 cols x 6 rows x 8 ch/reim fp16).  Visibilities are binned
    by (strip, col%8) into 16 streams.
  - DVE does a masked 36-tap multiply (f32 weights; host pre-multiplied
    wv*wu/2^23) + reduce into (4ch x re/im) f32 outputs.
fp16 is safe here only for the slab values (rounding is amplified ~16x by
interp cancellation: 2^-11*16 ~ 8e-3 < 2e-2); weights/products/DFT
constants must stay f32.
"""
import os
import sys

for _p in ("/opt/trn_rl_repo",):
    if _p not in sys.path and os.path.isdir(_p):
        sys.path.insert(0, _p)

import numpy as np

# ---- problem constants (must match reference.py) ----
NCH = 4
NPIX = 1024
NVIS = 200_000
G = 2048
J = 6
OSF = 2
CELL_ARCSEC = 0.005
DL = CELL_ARCSEC * np.pi / (180.0 * 3600.0)
BETA = float(np.pi * np.sqrt((J / OSF) ** 2 * (OSF - 0.5) ** 2 - 0.8))

# ---- sharding geometry ----
N_CORES = 8
P = 128
ROW_LO_ALL = -398            # min possible m0v
ROWS_PER_CORE = 100
R_ROWS = 105                 # slab rows per core (100 + 5 halo)
KU = 804                     # slab cols, freq COL_BASE + j
COL_BASE = -401
N1 = 256                     # stage-1 rhs width (2*105 used; f32r needs >=256)
STRIPW = 408                 # strip width in cols
SOFF = (0, 396)              # strip col offsets (windows never straddle)

CELL_E = J * 8               # 48 fp16 per B cell (6 rows x 4ch x re/im)
DESC_E = 8 * CELL_E          # 384 fp16 = 768B per gather descriptor (8 cols)
ROW_E = STRIPW * CELL_E      # 19584 fp16 per B row
UNITS_ROW = ROW_E // DESC_E  # 51 descriptor units per B row
NRES = 8
N_STREAMS = 2 * NRES         # 16 streams = (strip, col residue)
BROWS = ROWS_PER_CORE        # 100 B rows (window row starts)
CALL_IDX = 1024              # max gather descriptors per dma_gather call

SC = float(2.0 ** 23)        # fp16 slab scale (slab absmax*SC ~ 41)

C1 = np.float32(1000.0 * 2.0 * np.pi * DL)   # klambda -> rad/pixel
C2 = np.float32(G / (2.0 * np.pi))           # rad/pixel -> grid coord

V_SLOTS = None               # set by host_prep (= sum of per-stream slots)
S_ST = None                  # per-stream slot counts [16]
_NC_CACHE = {}


def build_nc(s_st=None):
    """Build the SPMD Bass program (same program for all 8 cores)."""
    if s_st is None:
        s_st = S_ST
    s_st = tuple(s_st)
    if s_st in _NC_CACHE:
        return _NC_CACHE[s_st]

    import concourse.bacc as bacc
    import concourse.mybir as mybir
    import concourse.tile as tile
    from contextlib import ExitStack

    SMAX = max(s_st)
    VS = sum(s_st)
    OFF = np.concatenate([[0], np.cumsum(s_st)]).astype(int)
    f32 = mybir.dt.float32
    f32r = mybir.dt.float32r
    fp16 = mybir.dt.float16
    i16 = mybir.dt.int16
    COPY = mybir.ActivationFunctionType.Copy

    nc = bacc.Bacc("TRN2", target_bir_lowering=False, debug=False)

    cube_d = nc.dram_tensor("cube", (NCH, NPIX, NPIX), f32r,
                            kind="ExternalInput")
    cvt_d = nc.dram_tensor("cvt", (P, 8, N1), f32r, kind="ExternalInput")
    cut_d = nc.dram_tensor("cut", (P, 8, KU), f32r, kind="ExternalInput")
    sut_d = nc.dram_tensor("sut", (P, 8, KU), f32r, kind="ExternalInput")
    shm_d = nc.dram_tensor("shm", (P, 5, BROWS), fp16, kind="ExternalInput")
    gidx_d = nc.dram_tensor("gidx", (P, 8 * VS), i16, kind="ExternalInput")
    w36_d = nc.dram_tensor("w36", (P, VS, 36), f32, kind="ExternalInput")
    out_d = nc.dram_tensor("vis_out", (P, VS, 8), f32, kind="ExternalOutput")
    b_d = [nc.dram_tensor(f"bscratch{i}", (BROWS, ROW_E), fp16)
           for i in range(2)]

    with tile.TileContext(nc) as tc:
        with ExitStack() as s12:
            tpool = s12.enter_context(tc.tile_pool(name="tmats", bufs=1))
            gpool = s12.enter_context(tc.tile_pool(name="grid", bufs=2))
            bpool = s12.enter_context(tc.tile_pool(name="bsb", bufs=1))
            ipool = s12.enter_context(tc.tile_pool(name="interp", bufs=2))
            psum_pool = s12.enter_context(
                tc.tile_pool(name="ps", bufs=8, space="PSUM"))
            s1z = ExitStack()
            const_pool = s1z.enter_context(tc.tile_pool(name="const", bufs=1))
            cube_pool = s1z.enter_context(tc.tile_pool(name="cube", bufs=3))
            cpool = s1z.enter_context(tc.tile_pool(name="cstream", bufs=4))

            cvt_sb = const_pool.tile([P, 8, N1], f32r)
            nc.sync.dma_start(cvt_sb[:], cvt_d[:])
            shm_sb = const_pool.tile([P, 5, BROWS], fp16)
            nc.sync.dma_start(shm_sb[:], shm_d[:])

            # T storage: (p, chan, term[T1,T2,negT1], xc, r)
            tall = tpool.tile([P, NCH, 3, 8, R_ROWS], f32r)

            # ---- stage 1: T^T = cube^T . cvt (accumulate over y chunks) ----
            for c in range(NCH):
                ps = [psum_pool.tile([P, N1], f32, tag="ps",
                                     name=f"ps1_{c}_{i}") for i in range(8)]
                for yc in range(8):
                    cb = cube_pool.tile([P, NPIX], f32r, tag="cube")
                    nc.sync.dma_start(cb[:], cube_d[c, yc * P:(yc + 1) * P, :])
                    for xt in range(8):
                        nc.tensor.matmul(
                            ps[xt][:],
                            lhsT=cb[:, xt * P:(xt + 1) * P],
                            rhs=cvt_sb[:, yc, :],
                            start=(yc == 0),
                            stop=(yc == 7),
                        )
                for xt in range(8):
                    src = ps[xt][:, 0:2 * R_ROWS].rearrange(
                        "p (t r) -> p t r", r=R_ROWS)
                    nc.scalar.activation(tall[:, c, 0:2, xt, :], src, COPY)
                nc.scalar.activation(tall[:, c, 2, :, :], tall[:, c, 0, :, :],
                                     COPY, scale=-1.0)

            # ---- stage 2a: both strips' slab matmuls -> fp16 grids ----
            grids = []
            for strip in range(2):
                off = SOFF[strip]
                ps2 = [psum_pool.tile([P, STRIPW], f32, tag="ps",
                                      name=f"ps2_{strip}_{i}")
                       for i in range(8)]  # (c, re/im) -> ps2[c*2+e]
                for xc in range(8):
                    cu = cpool.tile([P, STRIPW], f32r, tag="cu")
                    nc.sync.dma_start(cu[:], cut_d[:, xc, off:off + STRIPW])
                    su = cpool.tile([P, STRIPW], f32r, tag="su")
                    nc.sync.dma_start(su[:], sut_d[:, xc, off:off + STRIPW])
                    for c in range(NCH):
                        t1 = tall[:, c, 0, xc, :]
                        t2 = tall[:, c, 1, xc, :]
                        nt1 = tall[:, c, 2, xc, :]
                        # re = T1.cu + T2.su ; im = T2.cu + (-T1).su
                        nc.tensor.matmul(ps2[c * 2][:R_ROWS, :], lhsT=t1,
                                         rhs=cu[:], start=(xc == 0), stop=False)
                        nc.tensor.matmul(ps2[c * 2][:R_ROWS, :], lhsT=t2,
                                         rhs=su[:], start=False, stop=(xc == 7))
                        nc.tensor.matmul(ps2[c * 2 + 1][:R_ROWS, :], lhsT=t2,
                                         rhs=cu[:], start=(xc == 0), stop=False)
                        nc.tensor.matmul(ps2[c * 2 + 1][:R_ROWS, :], lhsT=nt1,
                                         rhs=su[:], start=False, stop=(xc == 7))
                grid_sb = gpool.tile([P, STRIPW * 8], fp16, tag="grid",
                                     name=f"grid_{strip}")
                gv = grid_sb[:].rearrange("p (c e) -> p c e", e=8)
                for c in range(NCH):
                    for e in range(2):
                        if (c * 2 + e) % 2 == 0:
                            nc.scalar.activation(gv[0:R_ROWS, :, c * 2 + e],
                                                 ps2[c * 2 + e][0:R_ROWS, :],
                                                 COPY, scale=SC)
                        else:
                            nc.vector.tensor_scalar_mul(
                                gv[0:R_ROWS, :, c * 2 + e],
                                ps2[c * 2 + e][0:R_ROWS, :], SC)
                grids.append(grid_sb)

            # ---- stage 2b: replicated-B build per strip + DMA out ----
            for strip in range(2):
                grid_sb = grids[strip]
                gv = grid_sb[:].rearrange("p (c e) -> p c e", e=8)
                b_sb = bpool.tile([P, ROW_E], fp16, tag="bsb",
                                  name=f"bsb_{strip}")
                bv = b_sb[:].rearrange("p (c i e) -> p c i e", i=J, e=8)
                nc.vector.tensor_copy(bv[0:BROWS, :, 0, :],
                                      gv[0:BROWS, :, :])
                for i in range(1, J):
                    for k in range(8):
                        pss = psum_pool.tile([P, STRIPW], f32, tag="ps",
                                             name=f"sh_{strip}_{i}_{k}")
                        nc.tensor.matmul(
                            pss[0:BROWS, :],
                            lhsT=shm_sb[0:R_ROWS, i - 1, :],
                            rhs=grid_sb[0:R_ROWS,
                                        k * STRIPW:(k + 1) * STRIPW],
                            start=True, stop=True)
                        dst = bv[0:BROWS, k * 51:(k + 1) * 51, i, :]
                        src = pss[0:BROWS, :].rearrange(
                            "p (c e) -> p c e", e=8)
                        if i % 2 == 0:
                            nc.vector.tensor_copy(dst, src)
                        else:
                            nc.scalar.activation(dst, src, COPY)
                nc.sync.dma_start(b_d[strip][:, :], b_sb[0:BROWS, :])

            # ---- stage 3: one 768B gather descriptor per visibility ----
            from concourse import library_config
            s1z.close()  # free stage-1/2-only SBUF zones for gw/ov
            gwpool = s12.enter_context(tc.tile_pool(name="gw", bufs=1))
            opool = s12.enter_context(tc.tile_pool(name="outp", bufs=1))

            nc.gpsimd.load_library(library_config.mlp)
            ov = opool.tile([P, VS, 8], f32)
            flats = [b_d[i][:, :].flatten() for i in range(2)]
            nv_max = (BROWS * ROW_E) // DESC_E  # descriptor units available
            for st in range(N_STREAMS):
                S = s_st[st]
                o0 = int(OFF[st])
                strip, rho = st // NRES, st % NRES
                nvu = nv_max if rho == 0 else nv_max - 1
                view = flats[strip][CELL_E * rho:
                                    CELL_E * rho + nvu * DESC_E].rearrange(
                    "(n e) -> n e", e=DESC_E)
                idxr = ipool.tile([P, 8 * SMAX], i16, tag="idx",
                                  name=f"idx_{st}")
                nc.sync.dma_start(
                    idxr[:, 0:8 * S], gidx_d[:, 8 * o0:8 * (o0 + S)])
                w = ipool.tile([P, SMAX, 36], f32, tag="w", name=f"w_{st}")
                nc.sync.dma_start(w[:, 0:S, :], w36_d[:, o0:o0 + S, :])
                g = ipool.tile([P, SMAX, DESC_E], fp16, tag="g",
                               name=f"g_{st}")
                done = 0
                while done < P * S:
                    n_idx = min(CALL_IDX, P * S - done)
                    nc.gpsimd.dma_gather(
                        out_ap=g[:, done // P:(done + n_idx) // P, :],
                        in_ap=view,
                        idxs_ap=idxr[:, done // 16:(done + n_idx) // 16],
                        num_idxs=n_idx,
                        num_idxs_reg=n_idx,
                        elem_size=DESC_E,
                        elem_step=DESC_E,
                    )
                    done += n_idx
                gw = gwpool.tile([P, SMAX, 36, 8], f32, tag="gw",
                                 name=f"gw_{st}")
                gm = g[:, 0:S, :].rearrange(
                    "p s (t e) -> p s t e", e=8)[:, :, 0:36, :]
                wb = w[:, 0:S, :].unsqueeze(3).to_broadcast([P, S, 36, 8])
                nc.vector.tensor_tensor(out=gw[:, 0:S], in0=gm, in1=wb,
                                        op=mybir.AluOpType.mult)
                rv = gw[:, 0:S].rearrange("p s t e -> p s e t")
                nc.vector.tensor_reduce(
                    out=ov[:, o0:o0 + S, :],
                    in_=rv,
                    axis=mybir.AxisListType.X,
                    op=mybir.AluOpType.add,
                )
            nc.sync.dma_start(out_d[:], ov[:])

    nc.compile()
    _NC_CACHE[s_st] = nc
    return nc


def _apod1d():
    f = np.arange(NPIX, dtype=np.float64) / G
    z = np.pi * J * f
    s = np.sqrt(BETA * BETA - z * z)
    return J * np.sinh(s) / s  # [NPIX] float64


def _interp_host(k):
    """Match reference _interp_coords index/weight math in f32."""
    t = (k.astype(np.float32) * C1) * C2
    m0 = np.floor(t).astype(np.int32)
    offs = np.arange(J, dtype=np.int32) - (J // 2 - 1)
    d = t[:, None] - (m0[:, None] + offs).astype(np.float32)
    w = np.i0(BETA * np.sqrt(np.maximum(0.0, 1.0 - (2.0 * d / J) ** 2)))
    return t, m0, w.astype(np.float32)


def host_prep(cube, uu, vv):
    """Returns (in_maps, meta, phase) for the 8 cores."""
    global V_SLOTS, S_ST
    cube = np.ascontiguousarray(np.asarray(cube, dtype=np.float32))
    uu = np.asarray(uu, dtype=np.float32)
    vv = np.asarray(vv, dtype=np.float32)

    s1 = _apod1d()
    y = np.arange(NPIX, dtype=np.float64)

    # u-direction DFT constants (same for all cores)
    kj = np.arange(KU, dtype=np.float64) + COL_BASE
    ang_u = 2.0 * np.pi * np.outer(y, kj) / G
    cut = (np.cos(ang_u) / s1[:, None]).astype(np.float32)
    sut = (np.sin(ang_u) / s1[:, None]).astype(np.float32)
    cut = np.ascontiguousarray(cut.reshape(8, P, KU).transpose(1, 0, 2))
    sut = np.ascontiguousarray(sut.reshape(8, P, KU).transpose(1, 0, 2))

    # shift matrices: shm[p, i, r] = 1 if p == r + i + 1
    shm = np.zeros((P, 5, BROWS), dtype=np.float16)
    for i in range(1, J):
        r = np.arange(BROWS)
        shm[r + i, i - 1, r] = 1.0

    tu, m0u, wu = _interp_host(uu)
    tv, m0v, wv = _interp_host(vv)
    assert m0u.min() >= -398 and m0u.max() < 398
    assert m0v.min() >= ROW_LO_ALL and m0v.max() < ROW_LO_ALL + 800

    core_of = (m0v - ROW_LO_ALL) // ROWS_PER_CORE
    j0 = m0u - 2 - COL_BASE        # window start col within slab, [1, 796]
    strip_of = (j0 > 400).astype(np.int64)
    colp = j0 - 396 * strip_of     # col within strip, [1,400] or [5,400]
    assert colp.min() >= 0 and colp.max() <= 400
    rho = colp % NRES
    q = colp // NRES               # descriptor unit within row, [0, 50]
    # [n, (c,i)] c-major: tap index t = c*6 + i
    w36 = (wv[:, :, None] * wu[:, None, :] / SC).astype(np.float32)
    w36 = np.ascontiguousarray(w36.transpose(0, 2, 1).reshape(-1, 36))

    # per-stream slots: max over cores of that stream's bin size
    orders = []
    S_ST = [1] * N_STREAMS
    for k in range(N_CORES):
        ok = []
        for st in range(N_STREAMS):
            sg, r = st // NRES, st % NRES
            order = np.where((core_of == k) & (strip_of == sg)
                             & (rho == r))[0]
            ok.append(order)
            S_ST[st] = max(S_ST[st], (len(order) + P - 1) // P)
        orders.append(ok)
    OFF = np.concatenate([[0], np.cumsum(S_ST)]).astype(int)
    V_SLOTS = int(OFF[-1])

    in_maps = []
    meta = []
    for k in range(N_CORES):
        row_lo = ROW_LO_ALL + ROWS_PER_CORE * k
        gidx = np.zeros((P, 8 * V_SLOTS), dtype=np.int16)
        w36k = np.zeros((P, V_SLOTS, 36), dtype=np.float32)
        meta_k = []
        for st in range(N_STREAMS):
            order = orders[k][st]
            S = S_ST[st]
            o0 = int(OFF[st])
            n = len(order)
            assert n <= P * S
            sl = np.arange(n)
            pp = sl % P
            vs = sl // P
            r0 = (m0v[order] - row_lo).astype(np.int64)   # [0, 100)
            assert n == 0 or (r0.min() >= 0 and r0.max() < BROWS)
            vals = r0 * UNITS_ROW + q[order]              # descriptor idx
            t = vs * P + pp
            block = np.zeros((16, 8 * S), dtype=np.int16)
            block[t % 16, t // 16] = vals.astype(np.int16)
            gidx[:, 8 * o0:8 * (o0 + S)] = np.tile(block, (8, 1))
            w36k[pp, o0 + vs, :] = w36[order]
            meta_k.append((order, pp, o0 + vs))
        # v-direction DFT constants for this core's rows
        kr = np.arange(R_ROWS, dtype=np.float64) + (row_lo - 2)
        ang_v = 2.0 * np.pi * np.outer(y, kr) / G
        blk = np.zeros((NPIX, N1), dtype=np.float32)
        blk[:, 0:R_ROWS] = np.cos(ang_v) / s1[:, None]
        blk[:, R_ROWS:2 * R_ROWS] = -np.sin(ang_v) / s1[:, None]
        cvt = np.ascontiguousarray(blk.reshape(8, P, N1).transpose(1, 0, 2))

        in_maps.append({
            "cube": cube,
            "cvt": cvt,
            "cut": cut,
            "sut": sut,
            "shm": shm,
            "gidx": gidx,
            "w36": w36k,
        })
        meta.append(meta_k)

    kv = vv * C1
    ku_ = uu * C1
    phase = np.exp(1j * (kv + ku_) * np.float32(NPIX / 2.0)).astype(
        np.complex64)
    return in_maps, meta, phase


def assemble(results, meta, phase):
    out = np.zeros((NCH, NVIS), dtype=np.complex64)
    for k in range(N_CORES):
        arr = results[k]["vis_out"].reshape(P, V_SLOTS, NCH, 2)
        for order, pp, rows in meta[k]:
            vals = arr[pp, rows]  # [n, NCH, 2]
            out[:, order] = (vals[..., 0] + 1j * vals[..., 1]).T
    return out * phase[None, :]


def kernel(cube, uu, vv):
    from concourse.bass_utils import run_bass_kernel_spmd

    in_maps, meta, phase = host_prep(cube, uu, vv)
    nc = build_nc()
    br = run_bass_kernel_spmd(
        nc, in_maps, list(range(N_CORES)),
        trace=bool(int(os.environ.get("NUFFT_TRACE", "0"))),
    )
    if br.exec_time_ns is not None:
        print(f"HW exec time: {br.exec_time_ns} ns")
    kernel.last_result = br
    return assemble(br.results, meta, phase)


# revision 15
# speedup vs baseline: 1.6765x; 1.1867x over previous
"""NuFFT forward (KbNufft-style) Trainium2 Bass kernel, v2.

Strategy (per core; vis sharded by v-row bin across 8 cores):
  - Stages 1-2 compute a 105-row x 804-col slab of the oversampled spectrum
    via partial-DFT matmuls (f32r, apodization folded into the constants):
        T = cube^T . [cos | -sin]   (per channel, 256-wide rhs for f32r rate)
        slab_re/im = T . [cut|sut]  (two 408-col strips)
  - The slab (scaled by 2^23) is stored fp16 and expanded into a 6-row
    replicated layout B[r][c] = rows r..r+5 of col c (48B cells) using 5
    shift-matmuls + Act/DVE interleave copies, then DMA'd to DRAM.
  - Each visibility's whole 6x6 KB window then becomes ONE 768B gather
    descriptor (8 cols x 6 rows x 8 ch/reim fp16).  Visibilities are binned
    by (strip, col%8) into 16 streams.
  - DVE does a masked 36-tap multiply (f32 weights; host pre-multiplied
    wv*wu/2^23) + reduce into (4ch x re/im) f32 outputs.
fp16 is safe here only for the slab values (rounding is amplified ~16x by
interp cancellation: 2^-11*16 ~ 8e-3 < 2e-2); weights/products/DFT
constants must stay f32.
"""
import os
import sys

for _p in ("/opt/trn_rl_repo",):
    if _p not in sys.path and os.path.isdir(_p):
        sys.path.insert(0, _p)

import numpy as np

# ---- problem constants (must match reference.py) ----
NCH = 4
NPIX = 1024
NVIS = 200_000
G = 2048
J = 6
OSF = 2
CELL_ARCSEC = 0.005
DL = CELL_ARCSEC * np.pi / (180.0 * 3600.0)
BETA = float(np.pi * np.sqrt((J / OSF) ** 2 * (OSF - 0.5) ** 2 - 0.8))

# ---- sharding geometry ----
N_CORES = 8
P = 128
ROW_LO_ALL = -398            # min possible m0v
ROWS_PER_CORE = 100
R_ROWS = 105                 # slab rows per core (100 + 5 halo)
KU = 804                     # slab cols, freq COL_BASE + j
COL_BASE = -401
N1 = 256                     # stage-1 rhs width (2*105 used; f32r needs >=256)
STRIPW = 408                 # strip width in cols
SOFF = (0, 396)              # strip col offsets (windows never straddle)

CELL_E = J * 8               # 48 fp16 per B cell (6 rows x 4ch x re/im)
DESC_E = 8 * CELL_E          # 384 fp16 = 768B per gather descriptor (8 cols)
ROW_E = STRIPW * CELL_E      # 19584 fp16 per B row
UNITS_ROW = ROW_E // DESC_E  # 51 descriptor units per B row
NRES = 8
N_STREAMS = 2 * NRES         # 16 streams = (strip, col residue)
BROWS = ROWS_PER_CORE        # 100 B rows (window row starts)
CALL_IDX = 1024              # max gather descriptors per dma_gather call

SC = float(2.0 ** 23)        # fp16 slab scale (slab absmax*SC ~ 41)

C1 = np.float32(1000.0 * 2.0 * np.pi * DL)   # klambda -> rad/pixel
C2 = np.float32(G / (2.0 * np.pi))           # rad/pixel -> grid coord

V_SLOTS = None               # set by host_prep (= sum of per-stream slots)
S_ST = None                  # per-stream slot counts [16]
_NC_CACHE = {}


def build_nc(s_st=None):
    """Build the SPMD Bass program (same program for all 8 cores)."""
    if s_st is None:
        s_st = S_ST
    s_st = tuple(s_st)
    if s_st in _NC_CACHE:
        return _NC_CACHE[s_st]

    import concourse.bacc as bacc
    import concourse.mybir as mybir
    import concourse.tile as tile
    from contextlib import ExitStack

    SMAX = max(s_st)
    VS = sum(s_st)
    OFF = np.concatenate([[0], np.cumsum(s_st)]).astype(int)
    f32 = mybir.dt.float32
    f32r = mybir.dt.float32r
    fp16 = mybir.dt.float16
    i16 = mybir.dt.int16
    COPY = mybir.ActivationFunctionType.Copy

    nc = bacc.Bacc("TRN2", target_bir_lowering=False, debug=False)

    cube_d = nc.dram_tensor("cube", (NCH, NPIX, NPIX), fp16,
                            kind="ExternalInput")
    cvt_d = nc.dram_tensor("cvt", (P, 8, N1), f32r, kind="ExternalInput")
    cut_d = nc.dram_tensor("cut", (P, 8, KU), f32r, kind="ExternalInput")
    sut_d = nc.dram_tensor("sut", (P, 8, KU), f32r, kind="ExternalInput")
    shm_d = nc.dram_tensor("shm", (P, 5, BROWS), fp16, kind="ExternalInput")
    gidx_d = nc.dram_tensor("gidx", (P, 8 * VS), i16, kind="ExternalInput")
    w36_d = nc.dram_tensor("w36", (P, VS, 36), f32, kind="ExternalInput")
    out_d = nc.dram_tensor("vis_out", (P, VS, 8), f32, kind="ExternalOutput")
    b_d = [nc.dram_tensor(f"bscratch{i}", (BROWS, ROW_E), fp16)
           for i in range(2)]

    with tile.TileContext(nc) as tc:
        with ExitStack() as s12:
            ipool = s12.enter_context(tc.tile_pool(name="interp", bufs=4))
            s2z = ExitStack()
            tpool = s2z.enter_context(tc.tile_pool(name="tmats", bufs=1))
            gpool = s2z.enter_context(tc.tile_pool(name="grid", bufs=2))
            bpool = s2z.enter_context(tc.tile_pool(name="bsb", bufs=1))
            psA = s12.enter_context(
                tc.tile_pool(name="psA", bufs=4, space="PSUM"))
            psB = s12.enter_context(
                tc.tile_pool(name="psB", bufs=4, space="PSUM"))
            s1z = ExitStack()
            const_pool = s1z.enter_context(tc.tile_pool(name="const", bufs=1))
            cube_pool = s1z.enter_context(tc.tile_pool(name="cube", bufs=3))
            cpool = s1z.enter_context(tc.tile_pool(name="cstream", bufs=8))

            cvt_sb = const_pool.tile([P, 8, N1], f32r)
            nc.sync.dma_start(cvt_sb[:], cvt_d[:])
            shm_sb = const_pool.tile([P, 5, BROWS], fp16)
            nc.sync.dma_start(shm_sb[:], shm_d[:])

            # T storage: (p, chan, term[T1,T2,negT1], xc, r)
            tall = tpool.tile([P, NCH, 3, 8, R_ROWS], f32r)

            # ---- stage 1: T^T = cube^T . cvt (accumulate over y chunks) ----
            for c in range(NCH):
                ps = [(psA if i < 4 else psB).tile([P, N1], f32, tag="ps",
                                                    name=f"ps1_{c}_{i}")
                      for i in range(8)]
                for yc in range(8):
                    cbh = cube_pool.tile([P, NPIX], fp16, tag="cubeh")
                    nc.sync.dma_start(cbh[:],
                                      cube_d[c, yc * P:(yc + 1) * P, :])
                    cb = cube_pool.tile([P, NPIX], f32r, tag="cube")
                    if yc % 2 == 0:
                        nc.scalar.activation(cb[:], cbh[:], COPY)
                    else:
                        nc.vector.tensor_copy(cb[:], cbh[:])
                    for xt in range(8):
                        nc.tensor.matmul(
                            ps[xt][:],
                            lhsT=cb[:, xt * P:(xt + 1) * P],
                            rhs=cvt_sb[:, yc, :],
                            start=(yc == 0),
                            stop=(yc == 7),
                        )
                for xt in range(8):
                    src = ps[xt][:, 0:2 * R_ROWS].rearrange(
                        "p (t r) -> p t r", r=R_ROWS)
                    nc.scalar.activation(tall[:, c, 0:2, xt, :], src, COPY)
                nc.scalar.activation(tall[:, c, 2, :, :], tall[:, c, 0, :, :],
                                     COPY, scale=-1.0)

            # ---- stage 2: per strip: slab matmuls -> fp16 grid -> B ----
            for strip in range(2):
                off = SOFF[strip]
                cus, sus = [], []
                for xc in range(8):
                    cu = cpool.tile([P, STRIPW], f32r, tag="cu")
                    nc.sync.dma_start(cu[:], cut_d[:, xc, off:off + STRIPW])
                    su = cpool.tile([P, STRIPW], f32r, tag="su")
                    nc.sync.dma_start(su[:], sut_d[:, xc, off:off + STRIPW])
                    cus.append(cu)
                    sus.append(su)
                grid_sb = gpool.tile([P, STRIPW * 8], fp16, tag="grid",
                                     name=f"grid_{strip}")
                gv = grid_sb[:].rearrange("p (c e) -> p c e", e=8)
                # re pass (psA), then im pass (psB)
                for half, pool in ((0, psA), (1, psB)):
                    ph = [pool.tile([P, STRIPW], f32, tag="ps",
                                    name=f"ps2_{strip}_{half}_{c}")
                          for c in range(NCH)]
                    for xc in range(8):
                        for c in range(NCH):
                            t1 = tall[:, c, 0, xc, :]
                            t2 = tall[:, c, 1, xc, :]
                            nt1 = tall[:, c, 2, xc, :]
                            # re = T1.cu + T2.su ; im = T2.cu + (-T1).su
                            la, lb = (t1, t2) if half == 0 else (t2, nt1)
                            nc.tensor.matmul(ph[c][:R_ROWS, :], lhsT=la,
                                             rhs=cus[xc][:],
                                             start=(xc == 0), stop=False)
                            nc.tensor.matmul(ph[c][:R_ROWS, :], lhsT=lb,
                                             rhs=sus[xc][:],
                                             start=False, stop=(xc == 7))
                    for c in range(NCH):
                        lane = c * 2 + half
                        if c % 2 == 0:
                            nc.scalar.activation(gv[0:R_ROWS, :, lane],
                                                 ph[c][0:R_ROWS, :],
                                                 COPY, scale=SC)
                        else:
                            nc.vector.tensor_scalar_mul(
                                gv[0:R_ROWS, :, lane],
                                ph[c][0:R_ROWS, :], SC)
                # replicated-B build
                b_sb = bpool.tile([P, ROW_E], fp16, tag="bsb",
                                  name=f"bsb_{strip}")
                bv = b_sb[:].rearrange("p (c i e) -> p c i e", i=J, e=8)
                nc.vector.tensor_copy(bv[0:BROWS, :, 0, :],
                                      gv[0:BROWS, :, :])
                for i in range(1, J):
                    for k in range(8):
                        pss = psA.tile([P, STRIPW], f32, tag="ps",
                                       name=f"sh_{strip}_{i}_{k}")
                        nc.tensor.matmul(
                            pss[0:BROWS, :],
                            lhsT=shm_sb[0:R_ROWS, i - 1, :],
                            rhs=grid_sb[0:R_ROWS,
                                        k * STRIPW:(k + 1) * STRIPW],
                            start=True, stop=True)
                        dst = bv[0:BROWS, k * 51:(k + 1) * 51, i, :]
                        src = pss[0:BROWS, :].rearrange(
                            "p (c e) -> p c e", e=8)
                        if i % 2 == 0:
                            nc.vector.tensor_copy(dst, src)
                        else:
                            nc.scalar.activation(dst, src, COPY)
                nc.sync.dma_start(b_d[strip][:, :], b_sb[0:BROWS, :])

            # ---- stage 3: one 768B gather descriptor per visibility ----
            from concourse import library_config
            s1z.close()  # free stage-1/2-only SBUF zones for gw/ov
            s2z.close()
            gwpool = s12.enter_context(tc.tile_pool(name="gw", bufs=1))
            opool = s12.enter_context(tc.tile_pool(name="outp", bufs=1))

            nc.gpsimd.load_library(library_config.mlp)
            ov = opool.tile([P, VS, 8], f32)
            flats = [b_d[i][:, :].flatten() for i in range(2)]
            nv_max = (BROWS * ROW_E) // DESC_E  # descriptor units available

            stream_parts = {}
            pending_red = {}

            def emit_mult(st):
                S, o0, g, w = stream_parts.pop(st)
                pool_mult = (st % 2 == 0)
                tag = "gwp" if pool_mult else "gw"
                gw = gwpool.tile([P, SMAX, 36, 8], f32, tag=tag,
                                 name=f"gw_{st}")
                gm = g[:, 0:S, :].rearrange(
                    "p s (t e) -> p s t e", e=8)[:, :, 0:36, :]
                wb = w[:, 0:S, :].unsqueeze(3).to_broadcast([P, S, 36, 8])
                eng = nc.gpsimd if pool_mult else nc.vector
                eng.tensor_tensor(out=gw[:, 0:S], in0=gm, in1=wb,
                                  op=mybir.AluOpType.mult)
                pending_red[st] = (S, o0, gw)

            def emit_reduce(st):
                S, o0, gw = pending_red.pop(st)
                rv = gw[:, 0:S].rearrange("p s t e -> p s e t")
                nc.vector.tensor_reduce(
                    out=ov[:, o0:o0 + S, :],
                    in_=rv,
                    axis=mybir.AxisListType.X,
                    op=mybir.AluOpType.add,
                )

            def flush_stream(st):
                emit_mult(st)
                emit_reduce(st)

            for st in range(N_STREAMS):
                S = s_st[st]
                o0 = int(OFF[st])
                strip, rho = st // NRES, st % NRES
                nvu = nv_max if rho == 0 else nv_max - 1
                view = flats[strip][CELL_E * rho:
                                    CELL_E * rho + nvu * DESC_E].rearrange(
                    "(n e) -> n e", e=DESC_E)
                idxr = ipool.tile([P, 8 * SMAX], i16, tag="idx",
                                  name=f"idx_{st}")
                nc.sync.dma_start(
                    idxr[:, 0:8 * S], gidx_d[:, 8 * o0:8 * (o0 + S)])
                w = ipool.tile([P, SMAX, 36], f32, tag="w", name=f"w_{st}")
                nc.sync.dma_start(w[:, 0:S, :], w36_d[:, o0:o0 + S, :])
                g = ipool.tile([P, SMAX, DESC_E], fp16, tag="g",
                               name=f"g_{st}")
                done = 0
                while done < P * S:
                    n_idx = min(CALL_IDX, P * S - done)
                    nc.gpsimd.dma_gather(
                        out_ap=g[:, done // P:(done + n_idx) // P, :],
                        in_ap=view,
                        idxs_ap=idxr[:, done // 16:(done + n_idx) // 16],
                        num_idxs=n_idx,
                        num_idxs_reg=n_idx,
                        elem_size=DESC_E,
                        elem_step=DESC_E,
                    )
                    done += n_idx
                stream_parts[st] = (S, o0, g, w)
                if st >= 2:
                    flush_stream(st - 2)
            flush_stream(N_STREAMS - 2)
            flush_stream(N_STREAMS - 1)
            for st in list(pending_red):
                emit_reduce(st)
            nc.sync.dma_start(out_d[:], ov[:])

    nc.compile()
    _NC_CACHE[s_st] = nc
    return nc


def _apod1d():
    f = np.arange(NPIX, dtype=np.float64) / G
    z = np.pi * J * f
    s = np.sqrt(BETA * BETA - z * z)
    return J * np.sinh(s) / s  # [NPIX] float64


def _interp_host(k):
    """Match reference _interp_coords index/weight math in f32."""
    t = (k.astype(np.float32) * C1) * C2
    m0 = np.floor(t).astype(np.int32)
    offs = np.arange(J, dtype=np.int32) - (J // 2 - 1)
    d = t[:, None] - (m0[:, None] + offs).astype(np.float32)
    w = np.i0(BETA * np.sqrt(np.maximum(0.0, 1.0 - (2.0 * d / J) ** 2)))
    return t, m0, w.astype(np.float32)


def host_prep(cube, uu, vv):
    """Returns (in_maps, meta, phase) for the 8 cores."""
    global V_SLOTS, S_ST
    cube = np.ascontiguousarray(np.asarray(cube, dtype=np.float32)).astype(np.float16)
    uu = np.asarray(uu, dtype=np.float32)
    vv = np.asarray(vv, dtype=np.float32)

    s1 = _apod1d()
    y = np.arange(NPIX, dtype=np.float64)

    # u-direction DFT constants (same for all cores)
    kj = np.arange(KU, dtype=np.float64) + COL_BASE
    ang_u = 2.0 * np.pi * np.outer(y, kj) / G
    cut = (np.cos(ang_u) / s1[:, None]).astype(np.float32)
    sut = (np.sin(ang_u) / s1[:, None]).astype(np.float32)
    cut = np.ascontiguousarray(cut.reshape(8, P, KU).transpose(1, 0, 2))
    sut = np.ascontiguousarray(sut.reshape(8, P, KU).transpose(1, 0, 2))

    # shift matrices: shm[p, i, r] = 1 if p == r + i + 1
    shm = np.zeros((P, 5, BROWS), dtype=np.float16)
    for i in range(1, J):
        r = np.arange(BROWS)
        shm[r + i, i - 1, r] = 1.0

    tu, m0u, wu = _interp_host(uu)
    tv, m0v, wv = _interp_host(vv)
    assert m0u.min() >= -398 and m0u.max() < 398
    assert m0v.min() >= ROW_LO_ALL and m0v.max() < ROW_LO_ALL + 800

    core_of = (m0v - ROW_LO_ALL) // ROWS_PER_CORE
    j0 = m0u - 2 - COL_BASE        # window start col within slab, [1, 796]
    strip_of = (j0 > 400).astype(np.int64)
    colp = j0 - 396 * strip_of     # col within strip, [1,400] or [5,400]
    assert colp.min() >= 0 and colp.max() <= 400
    rho = colp % NRES
    q = colp // NRES               # descriptor unit within row, [0, 50]
    # [n, (c,i)] c-major: tap index t = c*6 + i
    w36 = (wv[:, :, None] * wu[:, None, :] / SC).astype(np.float32)
    w36 = np.ascontiguousarray(w36.transpose(0, 2, 1).reshape(-1, 36))

    # per-stream slots: max over cores of that stream's bin size
    orders = []
    S_ST = [1] * N_STREAMS
    for k in range(N_CORES):
        ok = []
        for st in range(N_STREAMS):
            sg, r = st // NRES, st % NRES
            order = np.where((core_of == k) & (strip_of == sg)
                             & (rho == r))[0]
            ok.append(order)
            S_ST[st] = max(S_ST[st], (len(order) + P - 1) // P)
        orders.append(ok)
    OFF = np.concatenate([[0], np.cumsum(S_ST)]).astype(int)
    V_SLOTS = int(OFF[-1])

    in_maps = []
    meta = []
    for k in range(N_CORES):
        row_lo = ROW_LO_ALL + ROWS_PER_CORE * k
        gidx = np.zeros((P, 8 * V_SLOTS), dtype=np.int16)
        w36k = np.zeros((P, V_SLOTS, 36), dtype=np.float32)
        meta_k = []
        for st in range(N_STREAMS):
            order = orders[k][st]
            S = S_ST[st]
            o0 = int(OFF[st])
            n = len(order)
            assert n <= P * S
            sl = np.arange(n)
            pp = sl % P
            vs = sl // P
            r0 = (m0v[order] - row_lo).astype(np.int64)   # [0, 100)
            assert n == 0 or (r0.min() >= 0 and r0.max() < BROWS)
            vals = r0 * UNITS_ROW + q[order]              # descriptor idx
            t = vs * P + pp
            block = np.zeros((16, 8 * S), dtype=np.int16)
            block[t % 16, t // 16] = vals.astype(np.int16)
            gidx[:, 8 * o0:8 * (o0 + S)] = np.tile(block, (8, 1))
            w36k[pp, o0 + vs, :] = w36[order]
            meta_k.append((order, pp, o0 + vs))
        # v-direction DFT constants for this core's rows
        kr = np.arange(R_ROWS, dtype=np.float64) + (row_lo - 2)
        ang_v = 2.0 * np.pi * np.outer(y, kr) / G
        blk = np.zeros((NPIX, N1), dtype=np.float32)
        blk[:, 0:R_ROWS] = np.cos(ang_v) / s1[:, None]
        blk[:, R_ROWS:2 * R_ROWS] = -np.sin(ang_v) / s1[:, None]
        cvt = np.ascontiguousarray(blk.reshape(8, P, N1).transpose(1, 0, 2))

        in_maps.append({
            "cube": cube,
            "cvt": cvt,
            "cut": cut,
            "sut": sut,
            "shm": shm,
            "gidx": gidx,
            "w36": w36k,
        })
        meta.append(meta_k)

    kv = vv * C1
    ku_ = uu * C1
    phase = np.exp(1j * (kv + ku_) * np.float32(NPIX / 2.0)).astype(
        np.complex64)
    return in_maps, meta, phase


def assemble(results, meta, phase):
    out = np.zeros((NCH, NVIS), dtype=np.complex64)
    for k in range(N_CORES):
        arr = results[k]["vis_out"].reshape(P, V_SLOTS, NCH, 2)
        for order, pp, rows in meta[k]:
            vals = arr[pp, rows]  # [n, NCH, 2]
            out[:, order] = (vals[..., 0] + 1j * vals[..., 1]).T
    return out * phase[None, :]


def kernel(cube, uu, vv):
    from concourse.bass_utils import run_bass_kernel_spmd

    in_maps, meta, phase = host_prep(cube, uu, vv)
    nc = build_nc()
    br = run_bass_kernel_spmd(
        nc, in_maps, list(range(N_CORES)),
        trace=bool(int(os.environ.get("NUFFT_TRACE", "0"))),
    )
    if br.exec_time_ns is not None:
        print(f"HW exec time: {br.exec_time_ns} ns")
    kernel.last_result = br
    return assemble(br.results, meta, phase)


# revision 18
# speedup vs baseline: 1.7736x; 1.0579x over previous
"""NuFFT forward (KbNufft-style) Trainium2 Bass kernel, v2.

Strategy (per core; vis sharded by v-row bin across 8 cores):
  - Stages 1-2 compute a 105-row x 804-col slab of the oversampled spectrum
    via partial-DFT matmuls (f32r, apodization folded into the constants):
        T = cube^T . [cos | -sin]   (per channel, 256-wide rhs for f32r rate)
        slab_re/im = T . [cut|sut]  (two 408-col strips)
  - The slab (scaled by 2^23) is stored fp16 and expanded into a 6-row
    replicated layout B[r][c] = rows r..r+5 of col c (48B cells) using 5
    shift-matmuls + Act/DVE interleave copies, then DMA'd to DRAM.
  - Each visibility's whole 6x6 KB window then becomes ONE 768B gather
    descriptor (8 cols x 6 rows x 8 ch/reim fp16).  Visibilities are binned
    by (strip, col%8) into 16 streams.
  - DVE does a masked 36-tap multiply (f32 weights; host pre-multiplied
    wv*wu/2^23) + reduce into (4ch x re/im) f32 outputs.
fp16 is safe here only for the slab values (rounding is amplified ~16x by
interp cancellation: 2^-11*16 ~ 8e-3 < 2e-2); weights/products/DFT
constants must stay f32.
"""
import os
import sys

for _p in ("/opt/trn_rl_repo",):
    if _p not in sys.path and os.path.isdir(_p):
        sys.path.insert(0, _p)

import numpy as np

# ---- problem constants (must match reference.py) ----
NCH = 4
NPIX = 1024
NVIS = 200_000
G = 2048
J = 6
OSF = 2
CELL_ARCSEC = 0.005
DL = CELL_ARCSEC * np.pi / (180.0 * 3600.0)
BETA = float(np.pi * np.sqrt((J / OSF) ** 2 * (OSF - 0.5) ** 2 - 0.8))

# ---- sharding geometry ----
N_CORES = 8
P = 128
ROW_LO_ALL = -398            # min possible m0v
ROWS_PER_CORE = 100
R_ROWS = 105                 # slab rows per core (100 + 5 halo)
KU = 804                     # slab cols, freq COL_BASE + j
COL_BASE = -401
N1 = 256                     # stage-1 rhs width (2*105 used; f32r needs >=256)
STRIPW = 408                 # strip width in cols
SOFF = (0, 396)              # strip col offsets (windows never straddle)

CELL_E = J * 8               # 48 fp16 per B cell (6 rows x 4ch x re/im)
DESC_E = 8 * CELL_E          # 384 fp16 = 768B per gather descriptor (8 cols)
ROW_E = STRIPW * CELL_E      # 19584 fp16 per B row
UNITS_ROW = ROW_E // DESC_E  # 51 descriptor units per B row
NRES = 8
N_STREAMS = 2 * NRES         # 16 streams = (strip, col residue)
BROWS = ROWS_PER_CORE        # 100 B rows (window row starts)
CALL_IDX = 1024              # max gather descriptors per dma_gather call

SC = float(2.0 ** 23)        # fp16 slab scale (slab absmax*SC ~ 41)

C1 = np.float32(1000.0 * 2.0 * np.pi * DL)   # klambda -> rad/pixel
C2 = np.float32(G / (2.0 * np.pi))           # rad/pixel -> grid coord

V_SLOTS = None               # set by host_prep (= sum of per-stream slots)
S_ST = None                  # per-stream slot counts [16]
_NC_CACHE = {}


def build_nc(s_st=None):
    """Build the SPMD Bass program (same program for all 8 cores)."""
    if s_st is None:
        s_st = S_ST
    s_st = tuple(s_st)
    if s_st in _NC_CACHE:
        return _NC_CACHE[s_st]

    import concourse.bacc as bacc
    import concourse.mybir as mybir
    import concourse.tile as tile
    from contextlib import ExitStack

    SMAX = max(s_st)
    VS = sum(s_st)
    OFF = np.concatenate([[0], np.cumsum(s_st)]).astype(int)
    f32 = mybir.dt.float32
    f32r = mybir.dt.float32r
    fp16 = mybir.dt.float16
    i16 = mybir.dt.int16
    COPY = mybir.ActivationFunctionType.Copy

    nc = bacc.Bacc("TRN2", target_bir_lowering=False, debug=False)

    cube_d = nc.dram_tensor("cube", (NCH, NPIX, NPIX), fp16,
                            kind="ExternalInput")
    cvt_d = nc.dram_tensor("cvt", (P, 8, N1), f32r, kind="ExternalInput")
    cut_d = nc.dram_tensor("cut", (P, 8, KU), f32r, kind="ExternalInput")
    sut_d = nc.dram_tensor("sut", (P, 8, KU), f32r, kind="ExternalInput")
    shm_d = nc.dram_tensor("shm", (P, 5, BROWS), fp16, kind="ExternalInput")
    gidx_d = nc.dram_tensor("gidx", (P, 8 * VS), i16, kind="ExternalInput")
    w36_d = nc.dram_tensor("w36", (P, VS, 36), f32, kind="ExternalInput")
    out_d = nc.dram_tensor("vis_out", (P, VS, 8), f32, kind="ExternalOutput")
    b_d = [nc.dram_tensor(f"bscratch{i}", (BROWS, ROW_E), fp16)
           for i in range(2)]

    with tile.TileContext(nc) as tc:
        with ExitStack() as s12:
            ipool = s12.enter_context(tc.tile_pool(name="interp", bufs=4))
            s2z = ExitStack()
            tpool = s2z.enter_context(tc.tile_pool(name="tmats", bufs=1))
            gpool = s2z.enter_context(tc.tile_pool(name="grid", bufs=2))
            bpool = s2z.enter_context(tc.tile_pool(name="bsb", bufs=1))
            psA = s12.enter_context(
                tc.tile_pool(name="psA", bufs=4, space="PSUM"))
            psB = s12.enter_context(
                tc.tile_pool(name="psB", bufs=4, space="PSUM"))
            s1z = ExitStack()
            const_pool = s1z.enter_context(tc.tile_pool(name="const", bufs=1))
            cube_pool = s1z.enter_context(tc.tile_pool(name="cube", bufs=5))
            cpool = s1z.enter_context(tc.tile_pool(name="cstream", bufs=8))

            cvt_sb = const_pool.tile([P, 8, N1], f32r)
            nc.sync.dma_start(cvt_sb[:], cvt_d[:])
            shm_sb = const_pool.tile([P, 5, BROWS], fp16)
            nc.sync.dma_start(shm_sb[:], shm_d[:])

            # T storage: (p, chan, term[T1,T2,negT1], xc, r)
            tall = tpool.tile([P, NCH, 3, 8, R_ROWS], f32r)

            # ---- stage 1: T^T = cube^T . cvt (accumulate over y chunks) ----
            for c in range(NCH):
                ps = [(psA if i < 4 else psB).tile([P, N1], f32, tag="ps",
                                                    name=f"ps1_{c}_{i}")
                      for i in range(8)]
                for yc in range(8):
                    cbh = cube_pool.tile([P, NPIX], fp16, tag="cubeh")
                    nc.sync.dma_start(cbh[:],
                                      cube_d[c, yc * P:(yc + 1) * P, :])
                    cb = cube_pool.tile([P, NPIX], f32r, tag="cube")
                    if yc % 2 == 0:
                        nc.scalar.activation(cb[:], cbh[:], COPY)
                    else:
                        nc.vector.tensor_copy(cb[:], cbh[:])
                    for xt in range(8):
                        nc.tensor.matmul(
                            ps[xt][:],
                            lhsT=cb[:, xt * P:(xt + 1) * P],
                            rhs=cvt_sb[:, yc, :],
                            start=(yc == 0),
                            stop=(yc == 7),
                        )
                for xt in range(8):
                    src = ps[xt][:, 0:2 * R_ROWS].rearrange(
                        "p (t r) -> p t r", r=R_ROWS)
                    nc.scalar.activation(tall[:, c, 0:2, xt, :], src, COPY)
                nc.scalar.activation(tall[:, c, 2, :, :], tall[:, c, 0, :, :],
                                     COPY, scale=-1.0)

            # ---- stage 2: per strip: slab matmuls -> fp16 grid -> B ----
            for strip in range(2):
                off = SOFF[strip]
                cus, sus = [], []
                for xc in range(8):
                    cu = cpool.tile([P, STRIPW], f32r, tag="cu")
                    nc.sync.dma_start(cu[:], cut_d[:, xc, off:off + STRIPW])
                    su = cpool.tile([P, STRIPW], f32r, tag="su")
                    nc.sync.dma_start(su[:], sut_d[:, xc, off:off + STRIPW])
                    cus.append(cu)
                    sus.append(su)
                grid_sb = gpool.tile([P, STRIPW * 8], fp16, tag="grid",
                                     name=f"grid_{strip}")
                gv = grid_sb[:].rearrange("p (c e) -> p c e", e=8)
                # re pass (psA), then im pass (psB)
                for half, pool in ((0, psA), (1, psB)):
                    ph = [pool.tile([P, STRIPW], f32, tag="ps",
                                    name=f"ps2_{strip}_{half}_{c}")
                          for c in range(NCH)]
                    for xc in range(8):
                        for c in range(NCH):
                            t1 = tall[:, c, 0, xc, :]
                            t2 = tall[:, c, 1, xc, :]
                            nt1 = tall[:, c, 2, xc, :]
                            # re = T1.cu + T2.su ; im = T2.cu + (-T1).su
                            la, lb = (t1, t2) if half == 0 else (t2, nt1)
                            nc.tensor.matmul(ph[c][:R_ROWS, :], lhsT=la,
                                             rhs=cus[xc][:],
                                             start=(xc == 0), stop=False)
                            nc.tensor.matmul(ph[c][:R_ROWS, :], lhsT=lb,
                                             rhs=sus[xc][:],
                                             start=False, stop=(xc == 7))
                    for c in range(NCH):
                        lane = c * 2 + half
                        if c % 2 == 0:
                            nc.scalar.activation(gv[0:R_ROWS, :, lane],
                                                 ph[c][0:R_ROWS, :],
                                                 COPY, scale=SC)
                        else:
                            nc.vector.tensor_scalar_mul(
                                gv[0:R_ROWS, :, lane],
                                ph[c][0:R_ROWS, :], SC)
                # replicated-B build
                b_sb = bpool.tile([P, ROW_E], fp16, tag="bsb",
                                  name=f"bsb_{strip}")
                bv = b_sb[:].rearrange("p (c i e) -> p c i e", i=J, e=8)
                nc.vector.tensor_copy(bv[0:BROWS, :, 0, :],
                                      gv[0:BROWS, :, :])
                for i in range(1, J):
                    for k in range(8):
                        pss = psA.tile([P, STRIPW], f32, tag="ps",
                                       name=f"sh_{strip}_{i}_{k}")
                        nc.tensor.matmul(
                            pss[0:BROWS, :],
                            lhsT=shm_sb[0:R_ROWS, i - 1, :],
                            rhs=grid_sb[0:R_ROWS,
                                        k * STRIPW:(k + 1) * STRIPW],
                            start=True, stop=True)
                        dst = bv[0:BROWS, k * 51:(k + 1) * 51, i, :]
                        src = pss[0:BROWS, :].rearrange(
                            "p (c e) -> p c e", e=8)
                        if i % 2 == 0:
                            nc.vector.tensor_copy(dst, src)
                        else:
                            nc.scalar.activation(dst, src, COPY)
                nc.sync.dma_start(b_d[strip][:, :], b_sb[0:BROWS, :])

            # ---- stage 3: one 768B gather descriptor per visibility ----
            from concourse import library_config
            s1z.close()  # free stage-1/2-only SBUF zones for gw/ov
            s2z.close()
            gwpool = s12.enter_context(tc.tile_pool(name="gw", bufs=1))
            opool = s12.enter_context(tc.tile_pool(name="outp", bufs=1))

            nc.gpsimd.load_library(library_config.mlp)
            ov = opool.tile([P, VS, 8], f32)
            flats = [b_d[i][:, :].flatten() for i in range(2)]
            nv_max = (BROWS * ROW_E) // DESC_E  # descriptor units available

            stream_parts = {}
            pending_red = {}

            def emit_mult(st):
                S, o0, g, w = stream_parts.pop(st)
                pool_mult = (st % 2 == 0)
                tag = "gwp" if pool_mult else "gw"
                gw = gwpool.tile([P, SMAX, 36, 8], f32, tag=tag,
                                 name=f"gw_{st}")
                gm = g[:, 0:S, :].rearrange(
                    "p s (t e) -> p s t e", e=8)[:, :, 0:36, :]
                wb = w[:, 0:S, :].unsqueeze(3).to_broadcast([P, S, 36, 8])
                eng = nc.gpsimd if pool_mult else nc.vector
                eng.tensor_tensor(out=gw[:, 0:S], in0=gm, in1=wb,
                                  op=mybir.AluOpType.mult)
                pending_red[st] = (S, o0, gw)

            def emit_reduce(st):
                S, o0, gw = pending_red.pop(st)
                rv = gw[:, 0:S].rearrange("p s t e -> p s e t")
                nc.vector.tensor_reduce(
                    out=ov[:, o0:o0 + S, :],
                    in_=rv,
                    axis=mybir.AxisListType.X,
                    op=mybir.AluOpType.add,
                )

            def flush_stream(st):
                emit_mult(st)
                emit_reduce(st)

            for st in range(N_STREAMS):
                S = s_st[st]
                o0 = int(OFF[st])
                strip, rho = st // NRES, st % NRES
                nvu = nv_max if rho == 0 else nv_max - 1
                view = flats[strip][CELL_E * rho:
                                    CELL_E * rho + nvu * DESC_E].rearrange(
                    "(n e) -> n e", e=DESC_E)
                idxr = ipool.tile([P, 8 * SMAX], i16, tag="idx",
                                  name=f"idx_{st}")
                nc.sync.dma_start(
                    idxr[:, 0:8 * S], gidx_d[:, 8 * o0:8 * (o0 + S)])
                w = ipool.tile([P, SMAX, 36], f32, tag="w", name=f"w_{st}")
                nc.sync.dma_start(w[:, 0:S, :], w36_d[:, o0:o0 + S, :])
                g = ipool.tile([P, SMAX, DESC_E], fp16, tag="g",
                               name=f"g_{st}")
                done = 0
                while done < P * S:
                    n_idx = min(CALL_IDX, P * S - done)
                    nc.gpsimd.dma_gather(
                        out_ap=g[:, done // P:(done + n_idx) // P, :],
                        in_ap=view,
                        idxs_ap=idxr[:, done // 16:(done + n_idx) // 16],
                        num_idxs=n_idx,
                        num_idxs_reg=n_idx,
                        elem_size=DESC_E,
                        elem_step=DESC_E,
                    )
                    done += n_idx
                stream_parts[st] = (S, o0, g, w)
                if st >= 2:
                    flush_stream(st - 2)
            flush_stream(N_STREAMS - 2)
            flush_stream(N_STREAMS - 1)
            for st in list(pending_red):
                emit_reduce(st)
            nc.sync.dma_start(out_d[:], ov[:])

    nc.compile()
    _NC_CACHE[s_st] = nc
    return nc


def _apod1d():
    f = np.arange(NPIX, dtype=np.float64) / G
    z = np.pi * J * f
    s = np.sqrt(BETA * BETA - z * z)
    return J * np.sinh(s) / s  # [NPIX] float64


def _interp_host(k):
    """Match reference _interp_coords index/weight math in f32."""
    t = (k.astype(np.float32) * C1) * C2
    m0 = np.floor(t).astype(np.int32)
    offs = np.arange(J, dtype=np.int32) - (J // 2 - 1)
    d = t[:, None] - (m0[:, None] + offs).astype(np.float32)
    w = np.i0(BETA * np.sqrt(np.maximum(0.0, 1.0 - (2.0 * d / J) ** 2)))
    return t, m0, w.astype(np.float32)


def host_prep(cube, uu, vv):
    """Returns (in_maps, meta, phase) for the 8 cores."""
    global V_SLOTS, S_ST
    cube = np.ascontiguousarray(np.asarray(cube, dtype=np.float32)).astype(np.float16)
    uu = np.asarray(uu, dtype=np.float32)
    vv = np.asarray(vv, dtype=np.float32)

    s1 = _apod1d()
    y = np.arange(NPIX, dtype=np.float64)

    # u-direction DFT constants (same for all cores)
    kj = np.arange(KU, dtype=np.float64) + COL_BASE
    ang_u = 2.0 * np.pi * np.outer(y, kj) / G
    cut = (np.cos(ang_u) / s1[:, None]).astype(np.float32)
    sut = (np.sin(ang_u) / s1[:, None]).astype(np.float32)
    cut = np.ascontiguousarray(cut.reshape(8, P, KU).transpose(1, 0, 2))
    sut = np.ascontiguousarray(sut.reshape(8, P, KU).transpose(1, 0, 2))

    # shift matrices: shm[p, i, r] = 1 if p == r + i + 1
    shm = np.zeros((P, 5, BROWS), dtype=np.float16)
    for i in range(1, J):
        r = np.arange(BROWS)
        shm[r + i, i - 1, r] = 1.0

    tu, m0u, wu = _interp_host(uu)
    tv, m0v, wv = _interp_host(vv)
    assert m0u.min() >= -398 and m0u.max() < 398
    assert m0v.min() >= ROW_LO_ALL and m0v.max() < ROW_LO_ALL + 800

    core_of = (m0v - ROW_LO_ALL) // ROWS_PER_CORE
    j0 = m0u - 2 - COL_BASE        # window start col within slab, [1, 796]
    strip_of = (j0 > 400).astype(np.int64)
    colp = j0 - 396 * strip_of     # col within strip, [1,400] or [5,400]
    assert colp.min() >= 0 and colp.max() <= 400
    rho = colp % NRES
    q = colp // NRES               # descriptor unit within row, [0, 50]
    # [n, (c,i)] c-major: tap index t = c*6 + i
    w36 = (wv[:, :, None] * wu[:, None, :] / SC).astype(np.float32)
    w36 = np.ascontiguousarray(w36.transpose(0, 2, 1).reshape(-1, 36))

    # per-stream slots: max over cores of that stream's bin size
    orders = []
    S_ST = [1] * N_STREAMS
    for k in range(N_CORES):
        ok = []
        for st in range(N_STREAMS):
            sg, r = st // NRES, st % NRES
            order = np.where((core_of == k) & (strip_of == sg)
                             & (rho == r))[0]
            ok.append(order)
            S_ST[st] = max(S_ST[st], (len(order) + P - 1) // P)
        orders.append(ok)
    OFF = np.concatenate([[0], np.cumsum(S_ST)]).astype(int)
    V_SLOTS = int(OFF[-1])

    in_maps = []
    meta = []
    for k in range(N_CORES):
        row_lo = ROW_LO_ALL + ROWS_PER_CORE * k
        gidx = np.zeros((P, 8 * V_SLOTS), dtype=np.int16)
        w36k = np.zeros((P, V_SLOTS, 36), dtype=np.float32)
        meta_k = []
        for st in range(N_STREAMS):
            order = orders[k][st]
            S = S_ST[st]
            o0 = int(OFF[st])
            n = len(order)
            assert n <= P * S
            sl = np.arange(n)
            pp = sl % P
            vs = sl // P
            r0 = (m0v[order] - row_lo).astype(np.int64)   # [0, 100)
            assert n == 0 or (r0.min() >= 0 and r0.max() < BROWS)
            vals = r0 * UNITS_ROW + q[order]              # descriptor idx
            t = vs * P + pp
            block = np.zeros((16, 8 * S), dtype=np.int16)
            block[t % 16, t // 16] = vals.astype(np.int16)
            gidx[:, 8 * o0:8 * (o0 + S)] = np.tile(block, (8, 1))
            w36k[pp, o0 + vs, :] = w36[order]
            meta_k.append((order, pp, o0 + vs))
        # v-direction DFT constants for this core's rows
        kr = np.arange(R_ROWS, dtype=np.float64) + (row_lo - 2)
        ang_v = 2.0 * np.pi * np.outer(y, kr) / G
        blk = np.zeros((NPIX, N1), dtype=np.float32)
        blk[:, 0:R_ROWS] = np.cos(ang_v) / s1[:, None]
        blk[:, R_ROWS:2 * R_ROWS] = -np.sin(ang_v) / s1[:, None]
        cvt = np.ascontiguousarray(blk.reshape(8, P, N1).transpose(1, 0, 2))

        in_maps.append({
            "cube": cube,
            "cvt": cvt,
            "cut": cut,
            "sut": sut,
            "shm": shm,
            "gidx": gidx,
            "w36": w36k,
        })
        meta.append(meta_k)

    kv = vv * C1
    ku_ = uu * C1
    phase = np.exp(1j * (kv + ku_) * np.float32(NPIX / 2.0)).astype(
        np.complex64)
    return in_maps, meta, phase


def assemble(results, meta, phase):
    out = np.zeros((NCH, NVIS), dtype=np.complex64)
    for k in range(N_CORES):
        arr = results[k]["vis_out"].reshape(P, V_SLOTS, NCH, 2)
        for order, pp, rows in meta[k]:
            vals = arr[pp, rows]  # [n, NCH, 2]
            out[:, order] = (vals[..., 0] + 1j * vals[..., 1]).T
    return out * phase[None, :]


def kernel(cube, uu, vv):
    from concourse.bass_utils import run_bass_kernel_spmd

    in_maps, meta, phase = host_prep(cube, uu, vv)
    nc = build_nc()
    br = run_bass_kernel_spmd(
        nc, in_maps, list(range(N_CORES)),
        trace=bool(int(os.environ.get("NUFFT_TRACE", "0"))),
    )
    if br.exec_time_ns is not None:
        print(f"HW exec time: {br.exec_time_ns} ns")
    kernel.last_result = br
    return assemble(br.results, meta, phase)


# revision 20
# speedup vs baseline: 1.7825x; 1.0050x over previous
"""NuFFT forward (KbNufft-style) Trainium2 Bass kernel, v2.

Strategy (per core; vis sharded by v-row bin across 8 cores):
  - Stages 1-2 compute a 105-row x 804-col slab of the oversampled spectrum
    via partial-DFT matmuls (f32r, apodization folded into the constants):
        T = cube^T . [cos | -sin]   (per channel, 256-wide rhs for f32r rate)
        slab_re/im = T . [cut|sut]  (two 408-col strips)
  - The slab (scaled by 2^23) is stored fp16 and expanded into a 6-row
    replicated layout B[r][c] = rows r..r+5 of col c (48B cells) using 5
    shift-matmuls + Act/DVE interleave copies, then DMA'd to DRAM.
  - Each visibility's whole 6x6 KB window then becomes ONE 768B gather
    descriptor (8 cols x 6 rows x 8 ch/reim fp16).  Visibilities are binned
    by (strip, col%8) into 16 streams.
  - DVE does a masked 36-tap multiply (f32 weights; host pre-multiplied
    wv*wu/2^23) + reduce into (4ch x re/im) f32 outputs.
fp16 is safe here only for the slab values (rounding is amplified ~16x by
interp cancellation: 2^-11*16 ~ 8e-3 < 2e-2); weights/products/DFT
constants must stay f32.
"""
import os
import sys

for _p in ("/opt/trn_rl_repo",):
    if _p not in sys.path and os.path.isdir(_p):
        sys.path.insert(0, _p)

import numpy as np

# ---- problem constants (must match reference.py) ----
NCH = 4
NPIX = 1024
NVIS = 200_000
G = 2048
J = 6
OSF = 2
CELL_ARCSEC = 0.005
DL = CELL_ARCSEC * np.pi / (180.0 * 3600.0)
BETA = float(np.pi * np.sqrt((J / OSF) ** 2 * (OSF - 0.5) ** 2 - 0.8))

# ---- sharding geometry ----
N_CORES = 8
P = 128
ROW_LO_ALL = -398            # min possible m0v
ROWS_PER_CORE = 100
R_ROWS = 105                 # slab rows per core (100 + 5 halo)
KU = 804                     # slab cols, freq COL_BASE + j
COL_BASE = -401
N1 = 256                     # stage-1 rhs width (2*105 used; f32r needs >=256)
STRIPW = 408                 # strip width in cols
SOFF = (0, 396)              # strip col offsets (windows never straddle)

CELL_E = J * 8               # 48 fp16 per B cell (6 rows x 4ch x re/im)
DESC_E = 8 * CELL_E          # 384 fp16 = 768B per gather descriptor (8 cols)
ROW_E = STRIPW * CELL_E      # 19584 fp16 per B row
UNITS_ROW = ROW_E // DESC_E  # 51 descriptor units per B row
NRES = 8
N_STREAMS = 2 * NRES         # 16 streams = (strip, col residue)
BROWS = ROWS_PER_CORE        # 100 B rows (window row starts)
CALL_IDX = 1024              # max gather descriptors per dma_gather call

SC = float(2.0 ** 23)        # fp16 slab scale (slab absmax*SC ~ 41)

C1 = np.float32(1000.0 * 2.0 * np.pi * DL)   # klambda -> rad/pixel
C2 = np.float32(G / (2.0 * np.pi))           # rad/pixel -> grid coord

V_SLOTS = None               # set by host_prep (= sum of per-stream slots)
S_ST = None                  # per-stream slot counts [16]
_NC_CACHE = {}


def build_nc(s_st=None):
    """Build the SPMD Bass program (same program for all 8 cores)."""
    if s_st is None:
        s_st = S_ST
    s_st = tuple(s_st)
    if s_st in _NC_CACHE:
        return _NC_CACHE[s_st]

    import concourse.bacc as bacc
    import concourse.mybir as mybir
    import concourse.tile as tile
    from contextlib import ExitStack

    SMAX = max(s_st)
    VS = sum(s_st)
    OFF = np.concatenate([[0], np.cumsum(s_st)]).astype(int)
    f32 = mybir.dt.float32
    f32r = mybir.dt.float32r
    fp16 = mybir.dt.float16
    i16 = mybir.dt.int16
    COPY = mybir.ActivationFunctionType.Copy

    nc = bacc.Bacc("TRN2", target_bir_lowering=False, debug=False)

    cube_d = nc.dram_tensor("cube", (NCH, NPIX, NPIX), fp16,
                            kind="ExternalInput")
    cvt_d = nc.dram_tensor("cvt", (P, 8, N1), f32r, kind="ExternalInput")
    cut_d = nc.dram_tensor("cut", (P, 8, KU), f32r, kind="ExternalInput")
    sut_d = nc.dram_tensor("sut", (P, 8, KU), f32r, kind="ExternalInput")
    shm_d = nc.dram_tensor("shm", (P, 5, BROWS), fp16, kind="ExternalInput")
    gidx_d = nc.dram_tensor("gidx", (P, 8 * VS), i16, kind="ExternalInput")
    w36_d = nc.dram_tensor("w36", (P, VS, 36), f32, kind="ExternalInput")
    out_d = nc.dram_tensor("vis_out", (P, VS, 8), f32, kind="ExternalOutput")
    b_d = [nc.dram_tensor(f"bscratch{i}", (BROWS, ROW_E), fp16)
           for i in range(2)]

    with tile.TileContext(nc) as tc:
        with ExitStack() as s12:
            ipool = s12.enter_context(tc.tile_pool(name="interp", bufs=4))
            s2z = ExitStack()
            tpool = s2z.enter_context(tc.tile_pool(name="tmats", bufs=1))
            gpool = s2z.enter_context(tc.tile_pool(name="grid", bufs=2))
            bpool = s2z.enter_context(tc.tile_pool(name="bsb", bufs=1))
            psA = s12.enter_context(
                tc.tile_pool(name="psA", bufs=4, space="PSUM"))
            psB = s12.enter_context(
                tc.tile_pool(name="psB", bufs=4, space="PSUM"))
            s1z = ExitStack()
            const_pool = s1z.enter_context(tc.tile_pool(name="const", bufs=1))
            cube_pool = s1z.enter_context(tc.tile_pool(name="cube", bufs=5))
            cpool = s1z.enter_context(tc.tile_pool(name="cstream", bufs=8))

            cvt_sb = const_pool.tile([P, 8, N1], f32r)
            nc.sync.dma_start(cvt_sb[:], cvt_d[:])
            shm_sb = const_pool.tile([P, 5, BROWS], fp16)
            nc.sync.dma_start(shm_sb[:], shm_d[:])

            # T storage: (p, chan, term[T1,T2,negT1], xc, r)
            tall = tpool.tile([P, NCH, 3, 8, R_ROWS], f32r)

            # ---- stage 1: T^T = cube^T . cvt (accumulate over y chunks) ----
            for c in range(NCH):
                ps = [(psA if i < 4 else psB).tile([P, N1], f32, tag="ps",
                                                    name=f"ps1_{c}_{i}")
                      for i in range(8)]
                for yc in range(8):
                    cbh = cube_pool.tile([P, NPIX], fp16, tag="cubeh")
                    nc.sync.dma_start(cbh[:],
                                      cube_d[c, yc * P:(yc + 1) * P, :])
                    cb = cube_pool.tile([P, NPIX], f32r, tag="cube")
                    if yc % 2 == 0:
                        nc.scalar.activation(cb[:], cbh[:], COPY)
                    else:
                        nc.vector.tensor_copy(cb[:], cbh[:])
                    for xt in range(8):
                        nc.tensor.matmul(
                            ps[xt][:],
                            lhsT=cb[:, xt * P:(xt + 1) * P],
                            rhs=cvt_sb[:, yc, :],
                            start=(yc == 0),
                            stop=(yc == 7),
                        )
                for xt in range(8):
                    src = ps[xt][:, 0:2 * R_ROWS].rearrange(
                        "p (t r) -> p t r", r=R_ROWS)
                    nc.scalar.activation(tall[:, c, 0:2, xt, :], src, COPY)
                nc.scalar.activation(tall[:, c, 2, :, :], tall[:, c, 0, :, :],
                                     COPY, scale=-1.0)

            # ---- stage 2: per strip: slab matmuls -> fp16 grid -> B ----
            for strip in range(2):
                off = SOFF[strip]
                cus, sus = [], []
                for xc in range(8):
                    cu = cpool.tile([P, STRIPW], f32r, tag="cu")
                    nc.sync.dma_start(cu[:], cut_d[:, xc, off:off + STRIPW])
                    su = cpool.tile([P, STRIPW], f32r, tag="su")
                    nc.sync.dma_start(su[:], sut_d[:, xc, off:off + STRIPW])
                    cus.append(cu)
                    sus.append(su)
                grid_sb = gpool.tile([P, STRIPW * 8], fp16, tag="grid",
                                     name=f"grid_{strip}")
                gv = grid_sb[:].rearrange("p (c e) -> p c e", e=8)
                # re pass (psA), then im pass (psB)
                for half, pool in ((0, psA), (1, psB)):
                    ph = [pool.tile([P, STRIPW], f32, tag="ps",
                                    name=f"ps2_{strip}_{half}_{c}")
                          for c in range(NCH)]
                    for xc in range(8):
                        for c in range(NCH):
                            t1 = tall[:, c, 0, xc, :]
                            t2 = tall[:, c, 1, xc, :]
                            nt1 = tall[:, c, 2, xc, :]
                            # re = T1.cu + T2.su ; im = T2.cu + (-T1).su
                            la, lb = (t1, t2) if half == 0 else (t2, nt1)
                            nc.tensor.matmul(ph[c][:R_ROWS, :], lhsT=la,
                                             rhs=cus[xc][:],
                                             start=(xc == 0), stop=False)
                            nc.tensor.matmul(ph[c][:R_ROWS, :], lhsT=lb,
                                             rhs=sus[xc][:],
                                             start=False, stop=(xc == 7))
                    for c in range(NCH):
                        lane = c * 2 + half
                        if c % 2 == 0:
                            nc.scalar.activation(gv[0:R_ROWS, :, lane],
                                                 ph[c][0:R_ROWS, :],
                                                 COPY, scale=SC)
                        else:
                            nc.vector.tensor_scalar_mul(
                                gv[0:R_ROWS, :, lane],
                                ph[c][0:R_ROWS, :], SC)
                # replicated-B build
                b_sb = bpool.tile([P, ROW_E], fp16, tag="bsb",
                                  name=f"bsb_{strip}")
                bv = b_sb[:].rearrange("p (c i e) -> p c i e", i=J, e=8)
                nc.vector.tensor_copy(bv[0:BROWS, :, 0, :],
                                      gv[0:BROWS, :, :])
                for i in range(1, J):
                    for k in range(8):
                        pss = psA.tile([P, STRIPW], f32, tag="ps",
                                       name=f"sh_{strip}_{i}_{k}")
                        nc.tensor.matmul(
                            pss[0:BROWS, :],
                            lhsT=shm_sb[0:R_ROWS, i - 1, :],
                            rhs=grid_sb[0:R_ROWS,
                                        k * STRIPW:(k + 1) * STRIPW],
                            start=True, stop=True)
                        dst = bv[0:BROWS, k * 51:(k + 1) * 51, i, :]
                        src = pss[0:BROWS, :].rearrange(
                            "p (c e) -> p c e", e=8)
                        if i % 2 == 0:
                            nc.vector.tensor_copy(dst, src)
                        else:
                            nc.scalar.activation(dst, src, COPY)
                nc.sync.dma_start(b_d[strip][:, :], b_sb[0:BROWS, :])

            # ---- stage 3: one 768B gather descriptor per visibility ----
            from concourse import library_config
            s1z.close()  # free stage-1/2-only SBUF zones for gw/ov
            s2z.close()
            gwpool = s12.enter_context(tc.tile_pool(name="gw", bufs=1))
            opool = s12.enter_context(tc.tile_pool(name="outp", bufs=1))

            nc.gpsimd.load_library(library_config.mlp)
            ov = opool.tile([P, VS, 8], f32)
            flats = [b_d[i][:, :].flatten() for i in range(2)]
            nv_max = (BROWS * ROW_E) // DESC_E  # descriptor units available

            stream_parts = {}
            pending_red = {}

            def emit_mult(st):
                S, o0, g, w = stream_parts.pop(st)
                pool_mult = (st % 2 == 0)
                tag = "gwp" if pool_mult else "gw"
                gw = gwpool.tile([P, SMAX, 36, 8], f32, tag=tag,
                                 name=f"gw_{st}")
                gm = g[:, 0:S, :].rearrange(
                    "p s (t e) -> p s t e", e=8)[:, :, 0:36, :]
                wb = w[:, 0:S, :].unsqueeze(3).to_broadcast([P, S, 36, 8])
                eng = nc.gpsimd if pool_mult else nc.vector
                eng.tensor_tensor(out=gw[:, 0:S], in0=gm, in1=wb,
                                  op=mybir.AluOpType.mult)
                pending_red[st] = (S, o0, gw)

            def emit_reduce(st):
                S, o0, gw = pending_red.pop(st)
                rv = gw[:, 0:S].rearrange("p s t e -> p s e t")
                nc.vector.tensor_reduce(
                    out=ov[:, o0:o0 + S, :],
                    in_=rv,
                    axis=mybir.AxisListType.X,
                    op=mybir.AluOpType.add,
                )
                nc.sync.dma_start(out_d[:, o0:o0 + S, :],
                                  ov[:, o0:o0 + S, :])

            def flush_stream(st):
                emit_mult(st)
                emit_reduce(st)

            for st in range(N_STREAMS):
                S = s_st[st]
                o0 = int(OFF[st])
                strip, rho = st // NRES, st % NRES
                nvu = nv_max if rho == 0 else nv_max - 1
                view = flats[strip][CELL_E * rho:
                                    CELL_E * rho + nvu * DESC_E].rearrange(
                    "(n e) -> n e", e=DESC_E)
                idxr = ipool.tile([P, 8 * SMAX], i16, tag="idx",
                                  name=f"idx_{st}")
                nc.sync.dma_start(
                    idxr[:, 0:8 * S], gidx_d[:, 8 * o0:8 * (o0 + S)])
                w = ipool.tile([P, SMAX, 36], f32, tag="w", name=f"w_{st}")
                nc.sync.dma_start(w[:, 0:S, :], w36_d[:, o0:o0 + S, :])
                g = ipool.tile([P, SMAX, DESC_E], fp16, tag="g",
                               name=f"g_{st}")
                done = 0
                while done < P * S:
                    n_idx = min(CALL_IDX, P * S - done)
                    nc.gpsimd.dma_gather(
                        out_ap=g[:, done // P:(done + n_idx) // P, :],
                        in_ap=view,
                        idxs_ap=idxr[:, done // 16:(done + n_idx) // 16],
                        num_idxs=n_idx,
                        num_idxs_reg=n_idx,
                        elem_size=DESC_E,
                        elem_step=DESC_E,
                    )
                    done += n_idx
                stream_parts[st] = (S, o0, g, w)
                if st >= 2:
                    flush_stream(st - 2)
            flush_stream(N_STREAMS - 2)
            flush_stream(N_STREAMS - 1)
            for st in list(pending_red):
                emit_reduce(st)

    nc.compile()
    _NC_CACHE[s_st] = nc
    return nc


def _apod1d():
    f = np.arange(NPIX, dtype=np.float64) / G
    z = np.pi * J * f
    s = np.sqrt(BETA * BETA - z * z)
    return J * np.sinh(s) / s  # [NPIX] float64


def _interp_host(k):
    """Match reference _interp_coords index/weight math in f32."""
    t = (k.astype(np.float32) * C1) * C2
    m0 = np.floor(t).astype(np.int32)
    offs = np.arange(J, dtype=np.int32) - (J // 2 - 1)
    d = t[:, None] - (m0[:, None] + offs).astype(np.float32)
    w = np.i0(BETA * np.sqrt(np.maximum(0.0, 1.0 - (2.0 * d / J) ** 2)))
    return t, m0, w.astype(np.float32)


def host_prep(cube, uu, vv):
    """Returns (in_maps, meta, phase) for the 8 cores."""
    global V_SLOTS, S_ST
    cube = np.ascontiguousarray(np.asarray(cube, dtype=np.float32)).astype(np.float16)
    uu = np.asarray(uu, dtype=np.float32)
    vv = np.asarray(vv, dtype=np.float32)

    s1 = _apod1d()
    y = np.arange(NPIX, dtype=np.float64)

    # u-direction DFT constants (same for all cores)
    kj = np.arange(KU, dtype=np.float64) + COL_BASE
    ang_u = 2.0 * np.pi * np.outer(y, kj) / G
    cut = (np.cos(ang_u) / s1[:, None]).astype(np.float32)
    sut = (np.sin(ang_u) / s1[:, None]).astype(np.float32)
    cut = np.ascontiguousarray(cut.reshape(8, P, KU).transpose(1, 0, 2))
    sut = np.ascontiguousarray(sut.reshape(8, P, KU).transpose(1, 0, 2))

    # shift matrices: shm[p, i, r] = 1 if p == r + i + 1
    shm = np.zeros((P, 5, BROWS), dtype=np.float16)
    for i in range(1, J):
        r = np.arange(BROWS)
        shm[r + i, i - 1, r] = 1.0

    tu, m0u, wu = _interp_host(uu)
    tv, m0v, wv = _interp_host(vv)
    assert m0u.min() >= -398 and m0u.max() < 398
    assert m0v.min() >= ROW_LO_ALL and m0v.max() < ROW_LO_ALL + 800

    core_of = (m0v - ROW_LO_ALL) // ROWS_PER_CORE
    j0 = m0u - 2 - COL_BASE        # window start col within slab, [1, 796]
    strip_of = (j0 > 400).astype(np.int64)
    colp = j0 - 396 * strip_of     # col within strip, [1,400] or [5,400]
    assert colp.min() >= 0 and colp.max() <= 400
    rho = colp % NRES
    q = colp // NRES               # descriptor unit within row, [0, 50]
    # [n, (c,i)] c-major: tap index t = c*6 + i
    w36 = (wv[:, :, None] * wu[:, None, :] / SC).astype(np.float32)
    w36 = np.ascontiguousarray(w36.transpose(0, 2, 1).reshape(-1, 36))

    # per-stream slots: max over cores of that stream's bin size
    orders = []
    S_ST = [1] * N_STREAMS
    for k in range(N_CORES):
        ok = []
        for st in range(N_STREAMS):
            sg, r = st // NRES, st % NRES
            order = np.where((core_of == k) & (strip_of == sg)
                             & (rho == r))[0]
            ok.append(order)
            S_ST[st] = max(S_ST[st], (len(order) + P - 1) // P)
        orders.append(ok)
    OFF = np.concatenate([[0], np.cumsum(S_ST)]).astype(int)
    V_SLOTS = int(OFF[-1])

    in_maps = []
    meta = []
    for k in range(N_CORES):
        row_lo = ROW_LO_ALL + ROWS_PER_CORE * k
        gidx = np.zeros((P, 8 * V_SLOTS), dtype=np.int16)
        w36k = np.zeros((P, V_SLOTS, 36), dtype=np.float32)
        meta_k = []
        for st in range(N_STREAMS):
            order = orders[k][st]
            S = S_ST[st]
            o0 = int(OFF[st])
            n = len(order)
            assert n <= P * S
            sl = np.arange(n)
            pp = sl % P
            vs = sl // P
            r0 = (m0v[order] - row_lo).astype(np.int64)   # [0, 100)
            assert n == 0 or (r0.min() >= 0 and r0.max() < BROWS)
            vals = r0 * UNITS_ROW + q[order]              # descriptor idx
            t = vs * P + pp
            block = np.zeros((16, 8 * S), dtype=np.int16)
            block[t % 16, t // 16] = vals.astype(np.int16)
            gidx[:, 8 * o0:8 * (o0 + S)] = np.tile(block, (8, 1))
            w36k[pp, o0 + vs, :] = w36[order]
            meta_k.append((order, pp, o0 + vs))
        # v-direction DFT constants for this core's rows
        kr = np.arange(R_ROWS, dtype=np.float64) + (row_lo - 2)
        ang_v = 2.0 * np.pi * np.outer(y, kr) / G
        blk = np.zeros((NPIX, N1), dtype=np.float32)
        blk[:, 0:R_ROWS] = np.cos(ang_v) / s1[:, None]
        blk[:, R_ROWS:2 * R_ROWS] = -np.sin(ang_v) / s1[:, None]
        cvt = np.ascontiguousarray(blk.reshape(8, P, N1).transpose(1, 0, 2))

        in_maps.append({
            "cube": cube,
            "cvt": cvt,
            "cut": cut,
            "sut": sut,
            "shm": shm,
            "gidx": gidx,
            "w36": w36k,
        })
        meta.append(meta_k)

    kv = vv * C1
    ku_ = uu * C1
    phase = np.exp(1j * (kv + ku_) * np.float32(NPIX / 2.0)).astype(
        np.complex64)
    return in_maps, meta, phase


def assemble(results, meta, phase):
    out = np.zeros((NCH, NVIS), dtype=np.complex64)
    for k in range(N_CORES):
        arr = results[k]["vis_out"].reshape(P, V_SLOTS, NCH, 2)
        for order, pp, rows in meta[k]:
            vals = arr[pp, rows]  # [n, NCH, 2]
            out[:, order] = (vals[..., 0] + 1j * vals[..., 1]).T
    return out * phase[None, :]


def kernel(cube, uu, vv):
    from concourse.bass_utils import run_bass_kernel_spmd

    in_maps, meta, phase = host_prep(cube, uu, vv)
    nc = build_nc()
    br = run_bass_kernel_spmd(
        nc, in_maps, list(range(N_CORES)),
        trace=bool(int(os.environ.get("NUFFT_TRACE", "0"))),
    )
    if br.exec_time_ns is not None:
        print(f"HW exec time: {br.exec_time_ns} ns")
    kernel.last_result = br
    return assemble(br.results, meta, phase)


# revision 24
# speedup vs baseline: 1.8192x; 1.0206x over previous
"""NuFFT forward (KbNufft-style) Trainium2 Bass kernel, v2.

Strategy (per core; vis sharded by v-row bin across 8 cores):
  - Stages 1-2 compute a 105-row x 804-col slab of the oversampled spectrum
    via partial-DFT matmuls (f32r, apodization folded into the constants):
        T = cube^T . [cos | -sin]   (per channel, 256-wide rhs for f32r rate)
        slab_re/im = T . [cut|sut]  (two 408-col strips)
  - The slab (scaled by 2^23) is stored fp16 and expanded into a 6-row
    replicated layout B[r][c] = rows r..r+5 of col c (48B cells) using 5
    shift-matmuls + Act/DVE interleave copies, then DMA'd to DRAM.
  - Each visibility's whole 6x6 KB window then becomes ONE 768B gather
    descriptor (8 cols x 6 rows x 8 ch/reim fp16).  Visibilities are binned
    by (strip, col%8) into 16 streams.
  - DVE does a masked 36-tap multiply (f32 weights; host pre-multiplied
    wv*wu/2^23) + reduce into (4ch x re/im) f32 outputs.
fp16 is safe here only for the slab values (rounding is amplified ~16x by
interp cancellation: 2^-11*16 ~ 8e-3 < 2e-2); weights/products/DFT
constants must stay f32.
"""
import os
import sys

for _p in ("/opt/trn_rl_repo",):
    if _p not in sys.path and os.path.isdir(_p):
        sys.path.insert(0, _p)

import numpy as np

# ---- problem constants (must match reference.py) ----
NCH = 4
NPIX = 1024
NVIS = 200_000
G = 2048
J = 6
OSF = 2
CELL_ARCSEC = 0.005
DL = CELL_ARCSEC * np.pi / (180.0 * 3600.0)
BETA = float(np.pi * np.sqrt((J / OSF) ** 2 * (OSF - 0.5) ** 2 - 0.8))

# ---- sharding geometry ----
N_CORES = 8
P = 128
ROW_LO_ALL = -398            # min possible m0v
ROWS_PER_CORE = 100
R_ROWS = 105                 # slab rows per core (100 + 5 halo)
KU = 804                     # slab cols, freq COL_BASE + j
COL_BASE = -401
N1 = 256                     # stage-1 rhs width (2*105 used; f32r needs >=256)
STRIPW = 408                 # strip width in cols
SOFF = (0, 396)              # strip col offsets (windows never straddle)

CELL_E = J * 8               # 48 fp16 per B cell (6 rows x 4ch x re/im)
DESC_E = 8 * CELL_E          # 384 fp16 = 768B per gather descriptor (8 cols)
ROW_E = STRIPW * CELL_E      # 19584 fp16 per B row
UNITS_ROW = ROW_E // DESC_E  # 51 descriptor units per B row
NRES = 8
N_STREAMS = 2 * NRES         # 16 streams = (strip, col residue)
BROWS = ROWS_PER_CORE        # 100 B rows (window row starts)
CALL_IDX = 1024              # max gather descriptors per dma_gather call

SC = float(2.0 ** 23)        # fp16 slab scale (slab absmax*SC ~ 41)

C1 = np.float32(1000.0 * 2.0 * np.pi * DL)   # klambda -> rad/pixel
C2 = np.float32(G / (2.0 * np.pi))           # rad/pixel -> grid coord

V_SLOTS = None               # set by host_prep (= sum of per-stream slots)
S_ST = None                  # per-stream slot counts [16]
_NC_CACHE = {}


def build_nc(s_st=None):
    """Build the SPMD Bass program (same program for all 8 cores)."""
    if s_st is None:
        s_st = S_ST
    s_st = tuple(s_st)
    if s_st in _NC_CACHE:
        return _NC_CACHE[s_st]

    import concourse.bacc as bacc
    import concourse.mybir as mybir
    import concourse.tile as tile
    from contextlib import ExitStack

    SMAX = max(s_st)
    VS = sum(s_st)
    OFF = np.concatenate([[0], np.cumsum(s_st)]).astype(int)
    f32 = mybir.dt.float32
    f32r = mybir.dt.float32r
    fp16 = mybir.dt.float16
    i16 = mybir.dt.int16
    COPY = mybir.ActivationFunctionType.Copy

    nc = bacc.Bacc("TRN2", target_bir_lowering=False, debug=False)

    cube_d = nc.dram_tensor("cube", (NCH, NPIX, NPIX), fp16,
                            kind="ExternalInput")
    cvt_d = nc.dram_tensor("cvt", (P, 8, N1), f32r, kind="ExternalInput")
    cut_d = nc.dram_tensor("cut", (P, 8, KU), f32r, kind="ExternalInput")
    sut_d = nc.dram_tensor("sut", (P, 8, KU), f32r, kind="ExternalInput")
    shm_d = nc.dram_tensor("shm", (P, 5, BROWS), fp16, kind="ExternalInput")
    gidx_d = nc.dram_tensor("gidx", (P, 8 * VS), i16, kind="ExternalInput")
    w36_d = nc.dram_tensor("w36", (P, VS, 36), f32, kind="ExternalInput")
    out_d = nc.dram_tensor("vis_out", (P, VS, 8), f32, kind="ExternalOutput")
    b_d = [nc.dram_tensor(f"bscratch{i}", (BROWS, ROW_E), fp16)
           for i in range(2)]

    with tile.TileContext(nc) as tc:
        with ExitStack() as s12:
            ipool = s12.enter_context(tc.tile_pool(name="interp", bufs=4))
            s2z = ExitStack()
            tpool = s2z.enter_context(tc.tile_pool(name="tmats", bufs=1))
            gpool = s2z.enter_context(tc.tile_pool(name="grid", bufs=2))
            bpool = s2z.enter_context(tc.tile_pool(name="bsb", bufs=1))
            psA = s12.enter_context(
                tc.tile_pool(name="psA", bufs=4, space="PSUM"))
            psB = s12.enter_context(
                tc.tile_pool(name="psB", bufs=4, space="PSUM"))
            s1z = ExitStack()
            const_pool = s1z.enter_context(tc.tile_pool(name="const", bufs=1))
            cube_pool = s1z.enter_context(tc.tile_pool(name="cube", bufs=5))
            cpool = s1z.enter_context(tc.tile_pool(name="cstream", bufs=8))

            cvt_sb = const_pool.tile([P, 8, N1], f32r)
            nc.sync.dma_start(cvt_sb[:], cvt_d[:])
            shm_sb = const_pool.tile([P, 5, BROWS], fp16)
            nc.sync.dma_start(shm_sb[:], shm_d[:])

            # T storage: (p, chan, term[T1,T2,negT1], xc, r)
            tall = tpool.tile([P, NCH, 3, 8, R_ROWS], f32r)

            # ---- stage 1: T^T = cube^T . cvt (accumulate over y chunks) ----
            for c in range(NCH):
                ps = [(psA if i < 4 else psB).tile([P, N1], f32, tag="ps",
                                                    name=f"ps1_{c}_{i}")
                      for i in range(8)]
                for yc in range(8):
                    cbh = cube_pool.tile([P, NPIX], fp16, tag="cubeh")
                    nc.sync.dma_start(cbh[:],
                                      cube_d[c, yc * P:(yc + 1) * P, :])
                    cb = cube_pool.tile([P, NPIX], f32r, tag="cube")
                    nc.vector.tensor_copy(cb[:], cbh[:])
                    for xt in range(8):
                        nc.tensor.matmul(
                            ps[xt][:],
                            lhsT=cb[:, xt * P:(xt + 1) * P],
                            rhs=cvt_sb[:, yc, :],
                            start=(yc == 0),
                            stop=(yc == 7),
                        )
                for xt in range(8):
                    src = ps[xt][:, 0:2 * R_ROWS].rearrange(
                        "p (t r) -> p t r", r=R_ROWS)
                    if xt % 2 == 0:
                        nc.scalar.activation(tall[:, c, 0:2, xt, :], src,
                                             COPY)
                    else:
                        nc.vector.tensor_copy(tall[:, c, 0:2, xt, :], src)
                nc.scalar.activation(tall[:, c, 2, :, :], tall[:, c, 0, :, :],
                                     COPY, scale=-1.0)

            # ---- stage 2: per strip: slab matmuls -> fp16 grid -> B ----
            for strip in range(2):
                off = SOFF[strip]
                cus, sus = [], []
                for xc in range(8):
                    cu = cpool.tile([P, STRIPW], f32r, tag="cu")
                    nc.sync.dma_start(cu[:], cut_d[:, xc, off:off + STRIPW])
                    su = cpool.tile([P, STRIPW], f32r, tag="su")
                    nc.sync.dma_start(su[:], sut_d[:, xc, off:off + STRIPW])
                    cus.append(cu)
                    sus.append(su)
                grid_sb = gpool.tile([P, STRIPW * 8], fp16, tag="grid",
                                     name=f"grid_{strip}")
                gv = grid_sb[:].rearrange("p (c e) -> p c e", e=8)
                # re pass (psA), then im pass (psB)
                for half, pool in ((0, psA), (1, psB)):
                    ph = [pool.tile([P, STRIPW], f32, tag="ps",
                                    name=f"ps2_{strip}_{half}_{c}")
                          for c in range(NCH)]
                    for xc in range(8):
                        for c in range(NCH):
                            t1 = tall[:, c, 0, xc, :]
                            t2 = tall[:, c, 1, xc, :]
                            nt1 = tall[:, c, 2, xc, :]
                            # re = T1.cu + T2.su ; im = T2.cu + (-T1).su
                            la, lb = (t1, t2) if half == 0 else (t2, nt1)
                            nc.tensor.matmul(ph[c][:R_ROWS, :], lhsT=la,
                                             rhs=cus[xc][:],
                                             start=(xc == 0), stop=False)
                            nc.tensor.matmul(ph[c][:R_ROWS, :], lhsT=lb,
                                             rhs=sus[xc][:],
                                             start=False, stop=(xc == 7))
                    for c in range(NCH):
                        lane = c * 2 + half
                        if c % 2 == 0:
                            nc.scalar.activation(gv[0:R_ROWS, :, lane],
                                                 ph[c][0:R_ROWS, :],
                                                 COPY, scale=SC)
                        else:
                            nc.vector.tensor_scalar_mul(
                                gv[0:R_ROWS, :, lane],
                                ph[c][0:R_ROWS, :], SC)
                # replicated-B build
                b_sb = bpool.tile([P, ROW_E], fp16, tag="bsb",
                                  name=f"bsb_{strip}")
                bv = b_sb[:].rearrange("p (c i e) -> p c i e", i=J, e=8)
                nc.vector.tensor_copy(bv[0:BROWS, :, 0, :],
                                      gv[0:BROWS, :, :])
                for i in range(1, J):
                    for k in range(8):
                        pss = psA.tile([P, STRIPW], f32, tag="ps",
                                       name=f"sh_{strip}_{i}_{k}")
                        nc.tensor.matmul(
                            pss[0:BROWS, :],
                            lhsT=shm_sb[0:R_ROWS, i - 1, :],
                            rhs=grid_sb[0:R_ROWS,
                                        k * STRIPW:(k + 1) * STRIPW],
                            start=True, stop=True)
                        dst = bv[0:BROWS, k * 51:(k + 1) * 51, i, :]
                        src = pss[0:BROWS, :].rearrange(
                            "p (c e) -> p c e", e=8)
                        if i % 2 == 0:
                            nc.vector.tensor_copy(dst, src)
                        else:
                            nc.scalar.activation(dst, src, COPY)
                nc.sync.dma_start(b_d[strip][:, :], b_sb[0:BROWS, :])

            # ---- stage 3: one 768B gather descriptor per visibility ----
            from concourse import library_config
            s1z.close()  # free stage-1/2-only SBUF zones for gw/ov
            s2z.close()
            gwpool = s12.enter_context(tc.tile_pool(name="gw", bufs=1))
            opool = s12.enter_context(tc.tile_pool(name="outp", bufs=1))

            nc.gpsimd.load_library(library_config.mlp)
            ov = opool.tile([P, VS, 8], f32)
            flats = [b_d[i][:, :].flatten() for i in range(2)]
            nv_max = (BROWS * ROW_E) // DESC_E  # descriptor units available

            stream_parts = {}
            pending_red = {}

            def emit_mult(st):
                S, o0, g, w = stream_parts.pop(st)
                pool_mult = (st % 2 == 0)
                tag = "gwp" if pool_mult else "gw"
                gw = gwpool.tile([P, SMAX, 36, 8], f32, tag=tag,
                                 name=f"gw_{st}")
                gm = g[:, 0:S, :].rearrange(
                    "p s (t e) -> p s t e", e=8)[:, :, 0:36, :]
                wb = w[:, 0:S, :].unsqueeze(3).to_broadcast([P, S, 36, 8])
                eng = nc.gpsimd if pool_mult else nc.vector
                eng.tensor_tensor(out=gw[:, 0:S], in0=gm, in1=wb,
                                  op=mybir.AluOpType.mult)
                pending_red[st] = (S, o0, gw)

            def emit_reduce(st):
                S, o0, gw = pending_red.pop(st)
                rv = gw[:, 0:S].rearrange("p s t e -> p s e t")
                nc.vector.tensor_reduce(
                    out=ov[:, o0:o0 + S, :],
                    in_=rv,
                    axis=mybir.AxisListType.X,
                    op=mybir.AluOpType.add,
                )
                nc.sync.dma_start(out_d[:, o0:o0 + S, :],
                                  ov[:, o0:o0 + S, :])

            def flush_stream(st):
                emit_mult(st)
                emit_reduce(st)

            for st in range(N_STREAMS):
                S = s_st[st]
                o0 = int(OFF[st])
                strip, rho = st // NRES, st % NRES
                nvu = nv_max if rho == 0 else nv_max - 1
                view = flats[strip][CELL_E * rho:
                                    CELL_E * rho + nvu * DESC_E].rearrange(
                    "(n e) -> n e", e=DESC_E)
                idxr = ipool.tile([P, 8 * SMAX], i16, tag="idx",
                                  name=f"idx_{st}")
                nc.sync.dma_start(
                    idxr[:, 0:8 * S], gidx_d[:, 8 * o0:8 * (o0 + S)])
                w = ipool.tile([P, SMAX, 36], f32, tag="w", name=f"w_{st}")
                nc.sync.dma_start(w[:, 0:S, :], w36_d[:, o0:o0 + S, :])
                g = ipool.tile([P, SMAX, DESC_E], fp16, tag="g",
                               name=f"g_{st}")
                done = 0
                while done < P * S:
                    n_idx = min(CALL_IDX, P * S - done)
                    nc.gpsimd.dma_gather(
                        out_ap=g[:, done // P:(done + n_idx) // P, :],
                        in_ap=view,
                        idxs_ap=idxr[:, done // 16:(done + n_idx) // 16],
                        num_idxs=n_idx,
                        num_idxs_reg=n_idx,
                        elem_size=DESC_E,
                        elem_step=DESC_E,
                    )
                    done += n_idx
                stream_parts[st] = (S, o0, g, w)
                if st >= 2:
                    flush_stream(st - 2)
            flush_stream(N_STREAMS - 2)
            flush_stream(N_STREAMS - 1)
            for st in list(pending_red):
                emit_reduce(st)

    nc.compile()
    _NC_CACHE[s_st] = nc
    return nc


def _apod1d():
    f = np.arange(NPIX, dtype=np.float64) / G
    z = np.pi * J * f
    s = np.sqrt(BETA * BETA - z * z)
    return J * np.sinh(s) / s  # [NPIX] float64


def _interp_host(k):
    """Match reference _interp_coords index/weight math in f32."""
    t = (k.astype(np.float32) * C1) * C2
    m0 = np.floor(t).astype(np.int32)
    offs = np.arange(J, dtype=np.int32) - (J // 2 - 1)
    d = t[:, None] - (m0[:, None] + offs).astype(np.float32)
    w = np.i0(BETA * np.sqrt(np.maximum(0.0, 1.0 - (2.0 * d / J) ** 2)))
    return t, m0, w.astype(np.float32)


def host_prep(cube, uu, vv):
    """Returns (in_maps, meta, phase) for the 8 cores."""
    global V_SLOTS, S_ST
    cube = np.ascontiguousarray(np.asarray(cube, dtype=np.float32)).astype(np.float16)
    uu = np.asarray(uu, dtype=np.float32)
    vv = np.asarray(vv, dtype=np.float32)

    s1 = _apod1d()
    y = np.arange(NPIX, dtype=np.float64)

    # u-direction DFT constants (same for all cores)
    kj = np.arange(KU, dtype=np.float64) + COL_BASE
    ang_u = 2.0 * np.pi * np.outer(y, kj) / G
    cut = (np.cos(ang_u) / s1[:, None]).astype(np.float32)
    sut = (np.sin(ang_u) / s1[:, None]).astype(np.float32)
    cut = np.ascontiguousarray(cut.reshape(8, P, KU).transpose(1, 0, 2))
    sut = np.ascontiguousarray(sut.reshape(8, P, KU).transpose(1, 0, 2))

    # shift matrices: shm[p, i, r] = 1 if p == r + i + 1
    shm = np.zeros((P, 5, BROWS), dtype=np.float16)
    for i in range(1, J):
        r = np.arange(BROWS)
        shm[r + i, i - 1, r] = 1.0

    tu, m0u, wu = _interp_host(uu)
    tv, m0v, wv = _interp_host(vv)
    assert m0u.min() >= -398 and m0u.max() < 398
    assert m0v.min() >= ROW_LO_ALL and m0v.max() < ROW_LO_ALL + 800

    core_of = (m0v - ROW_LO_ALL) // ROWS_PER_CORE
    j0 = m0u - 2 - COL_BASE        # window start col within slab, [1, 796]
    strip_of = (j0 > 400).astype(np.int64)
    colp = j0 - 396 * strip_of     # col within strip, [1,400] or [5,400]
    assert colp.min() >= 0 and colp.max() <= 400
    rho = colp % NRES
    q = colp // NRES               # descriptor unit within row, [0, 50]
    # [n, (c,i)] c-major: tap index t = c*6 + i
    w36 = (wv[:, :, None] * wu[:, None, :] / SC).astype(np.float32)
    w36 = np.ascontiguousarray(w36.transpose(0, 2, 1).reshape(-1, 36))

    # per-stream slots: max over cores of that stream's bin size
    orders = []
    S_ST = [1] * N_STREAMS
    for k in range(N_CORES):
        ok = []
        for st in range(N_STREAMS):
            sg, r = st // NRES, st % NRES
            order = np.where((core_of == k) & (strip_of == sg)
                             & (rho == r))[0]
            ok.append(order)
            S_ST[st] = max(S_ST[st], (len(order) + P - 1) // P)
        orders.append(ok)
    OFF = np.concatenate([[0], np.cumsum(S_ST)]).astype(int)
    V_SLOTS = int(OFF[-1])

    in_maps = []
    meta = []
    for k in range(N_CORES):
        row_lo = ROW_LO_ALL + ROWS_PER_CORE * k
        gidx = np.zeros((P, 8 * V_SLOTS), dtype=np.int16)
        w36k = np.zeros((P, V_SLOTS, 36), dtype=np.float32)
        meta_k = []
        for st in range(N_STREAMS):
            order = orders[k][st]
            S = S_ST[st]
            o0 = int(OFF[st])
            n = len(order)
            assert n <= P * S
            sl = np.arange(n)
            pp = sl % P
            vs = sl // P
            r0 = (m0v[order] - row_lo).astype(np.int64)   # [0, 100)
            assert n == 0 or (r0.min() >= 0 and r0.max() < BROWS)
            vals = r0 * UNITS_ROW + q[order]              # descriptor idx
            t = vs * P + pp
            block = np.zeros((16, 8 * S), dtype=np.int16)
            block[t % 16, t // 16] = vals.astype(np.int16)
            gidx[:, 8 * o0:8 * (o0 + S)] = np.tile(block, (8, 1))
            w36k[pp, o0 + vs, :] = w36[order]
            meta_k.append((order, pp, o0 + vs))
        # v-direction DFT constants for this core's rows
        kr = np.arange(R_ROWS, dtype=np.float64) + (row_lo - 2)
        ang_v = 2.0 * np.pi * np.outer(y, kr) / G
        blk = np.zeros((NPIX, N1), dtype=np.float32)
        blk[:, 0:R_ROWS] = np.cos(ang_v) / s1[:, None]
        blk[:, R_ROWS:2 * R_ROWS] = -np.sin(ang_v) / s1[:, None]
        cvt = np.ascontiguousarray(blk.reshape(8, P, N1).transpose(1, 0, 2))

        in_maps.append({
            "cube": cube,
            "cvt": cvt,
            "cut": cut,
            "sut": sut,
            "shm": shm,
            "gidx": gidx,
            "w36": w36k,
        })
        meta.append(meta_k)

    kv = vv * C1
    ku_ = uu * C1
    phase = np.exp(1j * (kv + ku_) * np.float32(NPIX / 2.0)).astype(
        np.complex64)
    return in_maps, meta, phase


def assemble(results, meta, phase):
    out = np.zeros((NCH, NVIS), dtype=np.complex64)
    for k in range(N_CORES):
        arr = results[k]["vis_out"].reshape(P, V_SLOTS, NCH, 2)
        for order, pp, rows in meta[k]:
            vals = arr[pp, rows]  # [n, NCH, 2]
            out[:, order] = (vals[..., 0] + 1j * vals[..., 1]).T
    return out * phase[None, :]


def kernel(cube, uu, vv):
    from concourse.bass_utils import run_bass_kernel_spmd

    in_maps, meta, phase = host_prep(cube, uu, vv)
    nc = build_nc()
    br = run_bass_kernel_spmd(
        nc, in_maps, list(range(N_CORES)),
        trace=bool(int(os.environ.get("NUFFT_TRACE", "0"))),
    )
    if br.exec_time_ns is not None:
        print(f"HW exec time: {br.exec_time_ns} ns")
    kernel.last_result = br
    return assemble(br.results, meta, phase)
